# revision 1
# baseline (speedup 1.0000x reference)
"""HF OpenMoe attention (B=2,S=2048,HID=2048,NH=16,NKV=4,HD=128) on 8 trn2 cores.

Sharding: core c -> (batch b=c//4, kv-group g=c%4). Each core computes Q/K/V
projections for its 4 query heads + 1 kv head, RoPE, causal flash attention in
S^T layout (scores transposed: [k, q], softmax over the partition dim via
ones-matmul), and its partial o_proj; a 4-way ReduceScatter sums the o_proj
partials, each core returning a 512-row slice of o^T for its batch.

All matmuls run as float32r (full PE rate at N=512).
"""
import numpy as np
import concourse.bass as bass
import concourse.bacc as bacc
import concourse.tile as tile
import concourse.mybir as mybir
from concourse.bass_utils import run_bass_kernel_spmd
from concourse.masks import make_identity

f32 = mybir.dt.float32
f32r = mybir.dt.float32r
AF = mybir.ActivationFunctionType
MUL = mybir.AluOpType.mult
ADD = mybir.AluOpType.add

B, S, HID = 2, 2048, 2048
NH, NKV, HD = 16, 4, 128
GH = NH // NKV          # query heads per core (4)
TB = 512                # token block (q block / projection block)
NT = S // TB            # 4 token blocks
NCT = HID // 128        # 16 contraction tiles
NKT = S // 128          # 16 key tiles

_CACHE = {}


def _build(causal: bool, with_rs: bool = True):
    nc = bacc.Bacc("TRN2", target_bir_lowering=False, debug=False, num_devices=8)
    xt = nc.dram_tensor("xt", [HID, S], f32, kind="ExternalInput").ap()
    wq = nc.dram_tensor("wq", [HID, GH * HD], f32, kind="ExternalInput").ap()
    wk = nc.dram_tensor("wk", [HID, HD], f32, kind="ExternalInput").ap()
    wv = nc.dram_tensor("wv", [HID, HD], f32, kind="ExternalInput").ap()
    wo = nc.dram_tensor("wo", [GH * HD, HID], f32, kind="ExternalInput").ap()
    cos_d = nc.dram_tensor("cos_t", [HD, S], f32, kind="ExternalInput").ap()
    sin_d = nc.dram_tensor("sin_m", [HD, S], f32, kind="ExternalInput").ap()
    cm_d = nc.dram_tensor("cmask", [128, 4 * TB], f32, kind="ExternalInput").ap()
    id_d = nc.dram_tensor("ident_in", [128, 128], f32, kind="ExternalInput").ap()
    on_d = nc.dram_tensor("ones_in", [128, 128], f32, kind="ExternalInput").ap()
    out_r = nc.dram_tensor("out_r", [TB, S], f32, kind="ExternalOutput").ap()

    with tile.TileContext(nc) as tc:
        with (
            tc.tile_pool(name="glob", bufs=1) as glob,
            tc.tile_pool(name="dram", bufs=1, space="DRAM") as dram,
        ):
            # ---- global resident stores (65 KB/partition) ----
            kt_rope = glob.tile([128, S], f32r, tag="kt")          # roped K^T [d, k]
            v_all = glob.tile([128, S], f32r, tag="v")             # V natural, 128i:+128 = tile i
            qt_rope = [glob.tile([128, S], f32r, tag=f"q{h}", name=f"qt_rope{h}")
                       for h in range(GH)]
            ident = glob.tile([128, 128], f32r, tag="ident")
            nc.sync.dma_start(ident[:], id_d[:].bitcast(f32r))
            ones = glob.tile([128, 128], f32r, tag="ones")
            nc.sync.dma_start(ones[:], on_d[:].bitcast(f32r))

            oT_part = dram.tile([HID, S], f32)                     # o^T partial
            oT_red = dram.tile([TB, S], f32)

            # ---- phase A: projections + rope (phase-scoped SBUF) ----
            with tc.tile_pool(name="pA", bufs=1) as pA, \
                 tc.tile_pool(name="psA", bufs=1, space="PSUM") as psA:
                wq_all = pA.tile([128, NCT * GH * HD], f32r, tag="wq")   # [c-sub, ci*512+d]
                wk_all = pA.tile([128, NCT * HD], f32r, tag="wk")
                wv_all = pA.tile([128, NCT * HD], f32r, tag="wv")
                for ci in range(NCT):
                    cs_ = slice(128 * ci, 128 * (ci + 1))
                    nc.sync.dma_start(wq_all[:, ci * 512:(ci + 1) * 512], wq[cs_, :].bitcast(f32r))
                    nc.sync.dma_start(wk_all[:, ci * 128:(ci + 1) * 128], wk[cs_, :].bitcast(f32r))
                    nc.sync.dma_start(wv_all[:, ci * 128:(ci + 1) * 128], wv[cs_, :].bitcast(f32r))
                cos_s = pA.tile([128, S], f32, tag="cos")
                sin_s = pA.tile([128, S], f32, tag="sin")
                nc.sync.dma_start(cos_s[:], cos_d[:])
                nc.sync.dma_start(sin_s[:], sin_d[:])

                def rope(ps, dst_ap, tb):
                    """dst = ps*cos + swap64(ps)*sin_mod for token block tb."""
                    cs = cos_s[:, TB * tb:TB * (tb + 1)]
                    sn = sin_s[:, TB * tb:TB * (tb + 1)]
                    raw = pA.tile([128, TB], f32, tag="raw", bufs=3, name="raw")
                    nc.vector.tensor_copy(raw[:], ps[:])
                    rot = pA.tile([128, TB], f32, tag="rot", bufs=3, name="rot")
                    nc.sync.dma_start(rot[0:64, :], raw[64:128, :])
                    nc.sync.dma_start(rot[64:128, :], raw[0:64, :])
                    m1 = pA.tile([128, TB], f32, tag="m1", bufs=3, name="m1")
                    nc.vector.tensor_tensor(m1[:], raw[:], cs, op=MUL)
                    m2 = pA.tile([128, TB], f32, tag="m2", bufs=3, name="m2")
                    nc.vector.tensor_tensor(m2[:], rot[:], sn, op=MUL)
                    nc.vector.tensor_tensor(dst_ap, m1[:], m2[:], op=ADD)

                for tb in range(NT):
                    xt_t = []
                    for ci in range(NCT):
                        t = pA.tile([128, TB], f32r, tag="xt", bufs=16, name="xt")
                        nc.sync.dma_start(
                            t[:], xt[128 * ci:128 * (ci + 1),
                                     TB * tb:TB * (tb + 1)].bitcast(f32r))
                        xt_t.append(t)

                    ps_k = psA.tile([128, TB], f32, tag="pk")
                    ps_vt = psA.tile([128, TB], f32, tag="pv")
                    ps_q = [psA.tile([128, TB], f32, tag=f"pq{h}", name=f"ps_q{h}")
                            for h in range(GH)]
                    for ci in range(NCT):
                        st, sp = ci == 0, ci == NCT - 1
                        nc.tensor.matmul(ps_k[:], wk_all[:, ci * 128:(ci + 1) * 128],
                                         xt_t[ci][:], start=st, stop=sp)
                        nc.tensor.matmul(ps_vt[:], wv_all[:, ci * 128:(ci + 1) * 128],
                                         xt_t[ci][:], start=st, stop=sp)
                        for h in range(GH):
                            nc.tensor.matmul(ps_q[h][:],
                                             wq_all[:, ci * 512 + 128 * h:ci * 512 + 128 * (h + 1)],
                                             xt_t[ci][:], start=st, stop=sp)

                    rope(ps_k, kt_rope[:, TB * tb:TB * (tb + 1)], tb)
                    for h in range(GH):
                        rope(ps_q[h], qt_rope[h][:, TB * tb:TB * (tb + 1)], tb)

                    # V: V^T to sbuf, then PE-transpose 128-col pieces to natural layout
                    vt_sb = pA.tile([128, TB], f32r, tag="vts", bufs=2, name="vt_sb")
                    nc.vector.tensor_copy(vt_sb[:], ps_vt[:])
                    for u in range(TB // 128):
                        ps_tr = psA.tile([128, 128], f32r, tag="ptr", bufs=2, name="ps_tr")
                        nc.tensor.transpose(ps_tr[:], vt_sb[:, 128 * u:128 * (u + 1)], ident[:])
                        nc.vector.tensor_copy(
                            v_all[:, 128 * (4 * tb + u):128 * (4 * tb + u + 1)], ps_tr[:])

            # ---- phase B: attention + partial o_proj (phase-scoped SBUF) ----
            with tc.tile_pool(name="pB", bufs=1) as pB, \
                 tc.tile_pool(name="psB", bufs=1, space="PSUM") as psB:
                wo_all = pB.tile([128, GH * HID], f32r, tag="wo")  # [j-sub, jh*2048+c]
                for jh in range(GH):
                    nc.sync.dma_start(wo_all[:, jh * HID:(jh + 1) * HID],
                                      wo[128 * jh:128 * (jh + 1), :].bitcast(f32r))
                cm_s = pB.tile([128, 4 * TB], f32, tag="cm")
                nc.sync.dma_start(cm_s[:], cm_d[:])

                for j in range(NT):
                    nkt = 4 * (j + 1) if causal else NKT
                    ps_o = [psB.tile([128, TB], f32, tag=f"po{h}", name=f"ps_o{h}")
                            for h in range(GH)]
                    acc = [pB.tile([128, TB], f32r, tag=f"acc{h}", bufs=2, name=f"acch{h}")
                           for h in range(GH)]
                    for i in range(nkt):
                        m = i - 4 * j if causal else -1
                        for h in range(GH):
                            ps_s = psB.tile([128, TB], f32, tag="ps_s", bufs=2, name="ps_s")
                            nc.tensor.matmul(ps_s[:], kt_rope[:, 128 * i:128 * (i + 1)],
                                             qt_rope[h][:, TB * j:TB * (j + 1)],
                                             start=True, stop=True)
                            pt = pB.tile([128, TB], f32r, tag="pt", bufs=10, name="pt")
                            nc.scalar.activation(pt[:], ps_s[:], AF.Exp)
                            if 0 <= m:
                                pm = pB.tile([128, TB], f32r, tag="pm", bufs=4, name="pm")
                                nc.vector.tensor_tensor(
                                    pm[:], pt[:], cm_s[:, TB * m:TB * (m + 1)], op=MUL)
                                pt = pm
                            if i == 0:
                                nc.vector.tensor_copy(acc[h][:], pt[:])
                            else:
                                nc.vector.tensor_tensor(acc[h][:], acc[h][:], pt[:], op=ADD)
                            nc.tensor.matmul(ps_o[h][:],
                                             v_all[:, 128 * i:128 * (i + 1)], pt[:],
                                             start=(i == 0), stop=(i == nkt - 1))
                    # normalize into A^T blocks
                    at_s = [pB.tile([128, TB], f32r, tag=f"at{h}", bufs=2, name=f"at_s{h}")
                            for h in range(GH)]
                    for h in range(GH):
                        ps_d = psB.tile([128, TB], f32, tag="tmp", bufs=2, name="ps_d")
                        nc.tensor.matmul(ps_d[:], ones[:], acc[h][:], start=True, stop=True)
                        rec = pB.tile([128, TB], f32, tag="rec", bufs=4, name="rec")
                        nc.vector.reciprocal(rec[:], ps_d[:])
                        nc.vector.tensor_tensor(at_s[h][:], ps_o[h][:], rec[:], op=MUL)

                    # partial o_proj for this q block
                    for co in range(NCT):
                        ps_p = psB.tile([128, TB], f32, tag="tmp", bufs=2, name="ps_p")
                        for jh in range(GH):
                            nc.tensor.matmul(ps_p[:],
                                             wo_all[:, jh * HID + 128 * co:jh * HID + 128 * (co + 1)],
                                             at_s[jh][:], start=(jh == 0), stop=(jh == GH - 1))
                        ob = pB.tile([128, TB], f32, tag="ob", bufs=6, name="ob")
                        nc.vector.tensor_copy(ob[:], ps_p[:])
                        nc.sync.dma_start(
                            oT_part[128 * co:128 * (co + 1), TB * j:TB * (j + 1)], ob[:])

            # ---- phase C: ReduceScatter partials, emit this core's slice ----
            if with_rs:
                nc.gpsimd.collective_compute(
                    "ReduceScatter", ADD,
                    replica_groups=[[0, 1, 2, 3], [4, 5, 6, 7]],
                    ins=[oT_part[:].opt()], outs=[oT_red[:].opt()],
                )
                nc.sync.dma_start(out_r[:], oT_red[:])
            else:
                nc.sync.dma_start(out_r[:], oT_part[0:TB, :])

    nc.compile()
    return nc


def kernel(hidden_states, attention_mask, Wq, Wk, Wv, Wo, sin, cos):
    hidden_states = np.asarray(hidden_states, dtype=np.float32)
    attention_mask = np.asarray(attention_mask, dtype=np.float32)
    Wq, Wk, Wv, Wo = (np.ascontiguousarray(np.asarray(a, dtype=np.float32))
                      for a in (Wq, Wk, Wv, Wo))
    sin = np.asarray(sin, dtype=np.float32)
    cos = np.asarray(cos, dtype=np.float32)

    # classify the mask: causal (top-right strictly very-negative, elsewhere 0,
    # col 0 ignored since reference zeroes it) vs all-zeros (full attention)
    m0 = attention_mask[0, 0]
    iu = np.triu_indices(S, k=1)
    causal = bool((m0[iu] < -1e30).all() and
                  (m0[np.tril_indices(S, k=0)] == 0.0).all())
    if not causal:
        assert (attention_mask == 0).all(), "unsupported attention mask pattern"
    if causal:
        for b in range(1, B):
            assert np.array_equal(attention_mask[b, 0], m0), "mask differs per batch"

    key = causal
    if key not in _CACHE:
        _CACHE[key] = _build(causal)
    nc = _CACHE[key]

    cos_t = np.ascontiguousarray(cos[:S].T)          # [128, S]
    sin_t = cos_t.copy()
    sin_t[:] = sin[:S].T
    sin_m = sin_t.copy()
    sin_m[:64] *= -1.0
    # 0/1 causal keep-patterns for the 4 diagonal alignments
    kl = np.arange(128)[:, None]
    ql = np.arange(TB)[None, :]
    cmask = np.concatenate(
        [(ql >= kl + 128 * m).astype(np.float32) for m in range(4)], axis=1)

    in_maps = []
    for c in range(8):
        b, g = c // 4, c % 4
        in_maps.append({
            "xt": np.ascontiguousarray(hidden_states[b].T),
            "wq": np.ascontiguousarray(Wq[512 * g:512 * (g + 1), :].T),
            "wk": np.ascontiguousarray(Wk[128 * g:128 * (g + 1), :].T),
            "wv": np.ascontiguousarray(Wv[128 * g:128 * (g + 1), :].T),
            "wo": np.ascontiguousarray(Wo[:, 512 * g:512 * (g + 1)].T),
            "cos_t": cos_t, "sin_m": sin_m, "cmask": cmask,
            "ident_in": np.eye(128, dtype=np.float32),
            "ones_in": np.ones((128, 128), dtype=np.float32),
        })

    global _LAST_IN_MAPS
    _LAST_IN_MAPS = in_maps
    res = run_bass_kernel_spmd(nc, in_maps, core_ids=list(range(8)))

    out = np.empty((B, S, HID), dtype=np.float32)
    for c in range(8):
        b, r = c // 4, c % 4
        out[b, :, TB * r:TB * (r + 1)] = res.results[c]["out_r"].T
    return out


if __name__ == "__main__":
    rng = np.random.default_rng(0)
    h = rng.standard_normal((B, S, HID), dtype=np.float32)
    print("module loads ok")



# revision 49
# speedup vs baseline: 1.3293x; 1.3293x over previous
"""HF OpenMoe attention (B=2,S=2048,HID=2048,NH=16,NKV=4,HD=128) on 8 trn2 cores.

Sharding: core c -> (batch b=c//4, kv-group g=c%4). Each core computes Q/K/V
projections for its 4 query heads + 1 kv head, RoPE, causal flash attention in
S^T layout (scores transposed: [k, q], softmax over the partition dim via
ones-matmul), and its partial o_proj; a 4-way ReduceScatter sums the o_proj
partials, each core returning a 512-row slice of o^T for its batch.

Engine balance: PE does only matmuls (f32r for projections/scores/o_proj,
bf16 for P.V), exp runs on the scalar engine (bf16 out), elementwise work is
split between the vector (DVE) and pool (gpsimd) engines, rotate-half swaps
are SBUF DMAs on the pool queue, V transposes use the DMA xbar, and the causal
diagonal is tightened to 512/384/256/256-wide sub-tiles.
"""
import numpy as np
import concourse.bass as bass
import concourse.bacc as bacc
import concourse.tile as tile
import concourse.mybir as mybir
from concourse.bass_utils import run_bass_kernel_spmd

f32 = mybir.dt.float32
f32r = mybir.dt.float32r
bf16 = mybir.dt.bfloat16
AF = mybir.ActivationFunctionType
MUL = mybir.AluOpType.mult
ADD = mybir.AluOpType.add

B, S, HID = 2, 2048, 2048
NH, NKV, HD = 16, 4, 128
GH = NH // NKV          # query heads per core (4)
TB = 512                # token block (q block / projection block)
NT = S // TB            # 4 token blocks
NCT = HID // 128        # 16 contraction tiles
NKT = S // 128          # 16 key tiles

_CACHE = {}
_DEBUG_OUTS = False


def _build(causal: bool, with_rs: bool = True):
    nc = bacc.Bacc("TRN2", target_bir_lowering=False, debug=False, num_devices=8)
    xt = nc.dram_tensor("xt", [HID, S], f32, kind="ExternalInput").ap()
    wq = nc.dram_tensor("wq", [HID, GH * HD], f32, kind="ExternalInput").ap()
    wk = nc.dram_tensor("wk", [HID, HD], f32, kind="ExternalInput").ap()
    wv = nc.dram_tensor("wv", [HID, HD], f32, kind="ExternalInput").ap()
    wo = nc.dram_tensor("wo", [GH * HD, HID], f32, kind="ExternalInput").ap()
    cos_d = nc.dram_tensor("cos_t", [HD, S], f32, kind="ExternalInput").ap()
    sin_d = nc.dram_tensor("sin_m", [HD, S], f32, kind="ExternalInput").ap()
    cm_d = nc.dram_tensor("cmask", [128, 768], f32, kind="ExternalInput").ap()
    on_d = nc.dram_tensor("ones_in", [128, 128], f32, kind="ExternalInput").ap()
    id_d = nc.dram_tensor("ident_in", [128, 128], f32, kind="ExternalInput").ap()
    out_r = nc.dram_tensor("out_r", [TB, S], f32, kind="ExternalOutput").ap()

    with tile.TileContext(nc) as tc:
        with (
            tc.tile_pool(name="glob", bufs=1) as glob,
            tc.tile_pool(name="dram", bufs=1, space="DRAM") as dram,
        ):
            # ---- global resident stores ----
            kt_rope = glob.tile([128, S], f32r, tag="kt")          # roped K^T [d, k]
            v_all = glob.tile([128, S], bf16, tag="v")             # V natural (bf16), 128i:+128 = tile i
            qt_rope = [glob.tile([128, S], f32r, tag=f"q{h}", name=f"qt_rope{h}")
                       for h in range(GH)]
            cm_b = glob.tile([128, 768], bf16, tag="cmb")          # 0/1 keep masks (2 patterns)
            ones_b = glob.tile([128, 128], bf16, tag="onesb")
            ident_b = glob.tile([128, 128], bf16, tag="identb")

            oT_part = dram.tile([HID, S], f32)                     # o^T partial
            oT_red = dram.tile([TB, S], f32)

            # ---- phase A: projections + rope (phase-scoped SBUF) ----
            with tc.tile_pool(name="pA", bufs=1) as pA, \
                 tc.tile_pool(name="psA", bufs=1, space="PSUM") as psA:
                # batched weight loads: DRAM [c, d] -> SBUF [c-sub(128), ci, d].
                # Issue order matters: the DMA engines drain in order, so load
                # what phase A needs first (wk, rope tables), then xt(tb0) is
                # issued inside the loop, then wv/wq, and wo during tb1.
                wk_all = pA.tile([128, NCT, HD], f32r, tag="wk")
                nc.sync.dma_start(wk_all[:, 0:4, :], wk[0:512, :].bitcast(f32r)
                                  .rearrange("(c p) d -> p c d", p=128))
                id_f = pA.tile([128, 128], f32, tag="idf")
                nc.sync.dma_start(id_f[:], id_d[:])
                nc.vector.tensor_copy(ident_b[:], id_f[:])
                warm = pA.tile([128, 1], f32, tag="warm")
                nc.scalar.activation(warm[:], id_f[:, 0:1], AF.Exp)
                wv_all = pA.tile([128, NCT, HD], f32r, tag="wv")
                wq_all = [pA.tile([128, NCT, HD], f32r, tag=f"wqh{h}",
                                  name=f"wq_all{h}") for h in range(GH)]

                def rope(ps, dst_ap, cs, sn, flip):
                    """dst = ps*cos + swap64(ps)*sin_mod for token block tb."""
                    raw = pA.tile([128, TB], f32, tag="raw", bufs=2, name="raw")
                    nc.scalar.copy(raw[:], ps[:])
                    rot = pA.tile([128, TB], f32, tag="rot", bufs=6, name="rot")
                    nc.gpsimd.dma_start(rot[0:64, :], raw[64:128, :])
                    nc.gpsimd.dma_start(rot[64:128, :], raw[0:64, :])
                    m1 = pA.tile([128, TB], f32, tag="m1", bufs=6, name="m1")
                    nc.vector.tensor_tensor(m1[:], ps[:], cs[:], op=MUL)  # PSUM: DVE
                    nc.vector.tensor_tensor(rot[:], rot[:], sn[:], op=MUL)
                    nc.vector.tensor_tensor(dst_ap, m1[:], rot[:], op=ADD)

                for tb in range(NT):
                    cos_s = pA.tile([128, TB], f32, tag="cos", bufs=4, name="cos")
                    sin_s = pA.tile([128, TB], f32, tag="sin", bufs=4, name="sin")
                    if tb != 0:
                        # rope tables just ahead of the xt tiles
                        nc.sync.dma_start(cos_s[:], cos_d[:, TB * tb:TB * (tb + 1)])
                        nc.sync.dma_start(sin_s[:], sin_d[:, TB * tb:TB * (tb + 1)])
                    xt_t = []
                    for ci in range(NCT):
                        t = pA.tile([128, TB], f32r, tag="xt", bufs=32, name="xt")
                        nc.sync.dma_start(
                            t[:], xt[128 * ci:128 * (ci + 1),
                                     TB * tb:TB * (tb + 1)].bitcast(f32r))
                        xt_t.append(t)
                        if tb == 0 and ci % 4 == 3 and ci < 15:
                            c = ci // 4 + 1  # stream wk in behind the xt tiles
                            nc.sync.dma_start(
                                wk_all[:, 4 * c:4 * (c + 1), :],
                                wk[512 * c:512 * (c + 1), :].bitcast(f32r)
                                .rearrange("(c p) d -> p c d", p=128))
                        if tb == 0 and ci == 0:
                            nc.sync.dma_start(cos_s[:], cos_d[:, 0:TB])
                            nc.sync.dma_start(sin_s[:], sin_d[:, 0:TB])
                        if tb == 0 and ci == 10:
                            # wv lands right as the V matmuls want it; K's last
                            # xt tiles shift later but K ends DMA-paced anyway
                            nc.sync.dma_start(wv_all[:], wv[:].bitcast(f32r)
                                              .rearrange("(c p) d -> p c d", p=128))
                    if tb == 0:
                        for h in range(GH):
                            nc.sync.dma_start(
                                wq_all[h][:], wq[:, HD * h:HD * (h + 1)]
                                .bitcast(f32r).rearrange("(c p) d -> p c d", p=128))
                    # K
                    ps_k = psA.tile([128, TB], f32, tag="pk")
                    for ci in range(NCT):
                        nc.tensor.matmul(ps_k[:], wk_all[:, ci, :], xt_t[ci][:],
                                         start=ci == 0, stop=ci == NCT - 1)
                    rope(ps_k, kt_rope[:, TB * tb:TB * (tb + 1)], cos_s, sin_s,
                         flip=True)
                    # V
                    ps_v = psA.tile([128, TB], f32, tag="pv")
                    for ci in range(NCT):
                        nc.tensor.matmul(ps_v[:], wv_all[:, ci, :], xt_t[ci][:],
                                         start=ci == 0, stop=ci == NCT - 1)
                    vt_sb = pA.tile([128, TB], bf16, tag="vts", bufs=2, name="vt_sb")
                    nc.scalar.copy(vt_sb[:], ps_v[:])
                    for u in range(TB // 128):
                        ps_tr = psA.tile([128, 128], bf16, tag="ptr", bufs=2,
                                         name="ps_tr")
                        nc.tensor.transpose(ps_tr[:], vt_sb[:, 128 * u:128 * (u + 1)],
                                            ident_b[:])
                        # Act (not DVE): DVE's in-order queue sits behind rope
                        # m1 ops that can wait on the cos/sin loads
                        nc.scalar.copy(
                            v_all[:, 128 * (4 * tb + u):128 * (4 * tb + u + 1)],
                            ps_tr[:])
                    # Q heads
                    for h in range(GH):
                        ps_q = psA.tile([128, TB], f32, tag=f"pq{h}", name=f"ps_q{h}")
                        for ci in range(NCT):
                            nc.tensor.matmul(ps_q[:], wq_all[h][:, ci, :], xt_t[ci][:],
                                             start=ci == 0, stop=ci == NCT - 1)
                        rope(ps_q, qt_rope[h][:, TB * tb:TB * (tb + 1)], cos_s,
                             sin_s, flip=(h % 2 == 0))

            # ---- phase B: attention + partial o_proj (phase-scoped SBUF) ----
            with tc.tile_pool(name="pB", bufs=1) as pB, \
                 tc.tile_pool(name="psB", bufs=1, space="PSUM") as psB:
                # one-time small loads + casts (tiles scoped here, used here)
                cm_f = pB.tile([128, 768], f32, tag="cmf")
                nc.sync.dma_start(cm_f[:], cm_d[:])
                nc.vector.tensor_copy(cm_b[:], cm_f[:])
                on_f = pB.tile([128, 128], f32, tag="onf")
                nc.sync.dma_start(on_f[:], on_d[:])
                nc.vector.tensor_copy(ones_b[:], on_f[:])
                # o_proj weights: first o_proj runs ~25us into phase B, so this
                # load hides behind the first attention block
                wo_all = pB.tile([128, GH * HID], f32r, tag="wo")  # [j-sub, jh*2048+c]
                nc.sync.dma_start(wo_all[:].rearrange("p (h c) -> p h c", h=GH),
                                  wo[:].bitcast(f32r)
                                  .rearrange("(h p) c -> p h c", p=128))

                def emit_oproj(j, at_j):
                    """o_proj matmuls for q-block j (deferred one j for pipelining).

                    Output DMAs are batched 4 co-tiles at a time to keep the SP
                    queue short."""
                    for cb in range(NCT // 4):
                        ob4 = pB.tile([128, 4, TB], f32, tag="ob", bufs=2,
                                      name="ob4")
                        for u in range(4):
                            co = 4 * cb + u
                            ps_p = psB.tile([128, TB], f32, tag="ps_d", bufs=2,
                                            name="ps_p")
                            for jh in range(GH):
                                nc.tensor.matmul(ps_p[:],
                                                 wo_all[:, jh * HID + 128 * co:
                                                        jh * HID + 128 * (co + 1)],
                                                 at_j[jh][:], start=(jh == 0),
                                                 stop=(jh == GH - 1))
                            if u % 2 == 0:
                                nc.vector.tensor_copy(ob4[:, u, :], ps_p[:])
                            else:
                                nc.scalar.copy(ob4[:, u, :], ps_p[:])
                        if cb == 0 and not with_rs:
                            # no-collective mode returns oT_part[0:512] as
                            # out_r; write it there directly
                            dst = out_r[:, TB * j:TB * (j + 1)]
                        else:
                            dst = oT_part[512 * cb:512 * (cb + 1),
                                          TB * j:TB * (j + 1)]
                        nc.sync.dma_start(
                            dst.rearrange("(u p) t -> p u t", p=128), ob4[:])

                pending = None  # (j, at_s list) awaiting o_proj emission
                # q-blocks in rotated order so the final (tail) o_proj pairs
                # with the SHORT j=0 block: its at_s chain hides behind the
                # previous block's o_proj instead of stalling the PE
                for j in ([1, 2, 3, 0] if causal else range(NT)):
                    # (i, q-col-offset) tile list: full tiles then tightened diagonal
                    if causal:
                        tiles = [(i, 0) for i in range(4 * j)]
                        tiles += [(4 * j + m, min(128 * m, 256)) for m in range(4)]
                    else:
                        tiles = [(i, 0) for i in range(NKT)]
                    last_i = tiles[-1][0]

                    ps_o = [psB.tile([128, TB], f32, tag="po", bufs=4, name=f"ps_o{h}")
                            for h in range(GH)]
                    acc = [pB.tile([128, TB], bf16, tag=f"acc{h}", bufs=1,
                                   name=f"acch{h}") for h in range(GH)]
                    av_pend = []  # (h, i, off, w, pt) AVs deferred from i<2

                    def flush_av():
                        for (fh, fi, foff, fw, fpt) in av_pend:
                            nc.tensor.matmul(ps_o[fh][:, foff:TB],
                                             v_all[:, 128 * fi:128 * (fi + 1)],
                                             fpt[:, 0:fw],
                                             start=(fi == 0), stop=(fi == last_i),
                                             skip_group_check=True)
                        av_pend.clear()

                    for i, off in tiles:
                        w = TB - off
                        diag = causal and i >= 4 * j
                        m = i - 4 * j if diag else -1
                        if i == 2:
                            # the i<2 AVs were deferred so the first PSUM write
                            # to the po banks (WAR on last block's at_s) comes
                            # after ~8 scores' worth of PE work
                            flush_av()
                        for h in range(GH):
                            ps_s = psB.tile([128, TB], f32, tag="ps_s", bufs=2,
                                            name="ps_s")
                            nc.tensor.matmul(ps_s[:, 0:w],
                                             kt_rope[:, 128 * i:128 * (i + 1)],
                                             qt_rope[h][:, TB * j + off:TB * (j + 1)],
                                             start=True, stop=True)
                            pt = pB.tile([128, TB], bf16, tag="pt", bufs=8, name="pt")
                            nc.scalar.activation(pt[:, 0:w], ps_s[:, 0:w], AF.Exp)
                            if diag:
                                patt = 512 if m == 3 else 0
                                meng = nc.gpsimd if w <= 256 else nc.vector
                                meng.tensor_tensor(
                                    pt[:, 0:w], pt[:, 0:w],
                                    cm_b[:, patt:patt + w], op=MUL)
                            if i == 0:
                                nc.gpsimd.tensor_copy(acc[h][:], pt[:])
                            else:
                                nc.vector.tensor_tensor(acc[h][:, off:TB],
                                                        acc[h][:, off:TB],
                                                        pt[:, 0:w], op=ADD)
                            if i < 2:
                                av_pend.append((h, i, off, w, pt))
                                continue
                            nc.tensor.matmul(ps_o[h][:, off:TB],
                                             v_all[:, 128 * i:128 * (i + 1)],
                                             pt[:, 0:w],
                                             start=False, stop=(i == last_i),
                                             skip_group_check=True)
                    flush_av()
                    # o_proj of the previous q block (its at_s is ready by now;
                    # emitting it here keeps the PE fed while this block's
                    # den->rec->at_s chain drains on DVE)
                    if pending is not None:
                        emit_oproj(*pending)

                    # normalize into A^T blocks
                    at_s = [pB.tile([128, TB], f32r, tag=f"at{h}", bufs=1,
                                    name=f"at_s{h}") for h in range(GH)]
                    for h in range(GH):
                        ps_d = psB.tile([128, TB], f32, tag="ps_d", bufs=2,
                                        name="ps_d")
                        nc.tensor.matmul(ps_d[:], ones_b[:], acc[h][:],
                                         start=True, stop=True)
                        rec = pB.tile([128, TB], f32, tag="rec", bufs=2, name="rec")
                        nc.vector.reciprocal(rec[:], ps_d[:])
                        nc.vector.tensor_tensor(at_s[h][:], ps_o[h][:], rec[:],
                                                op=MUL)  # PSUM: DVE only
                    pending = (j, at_s)
                emit_oproj(*pending)

            if _DEBUG_OUTS:
                dbg_kt = nc.dram_tensor("dbg_kt", [128, S], f32,
                                        kind="ExternalOutput").ap()
                dbg_q0 = nc.dram_tensor("dbg_q0", [128, S], f32,
                                        kind="ExternalOutput").ap()
                dbg_v = nc.dram_tensor("dbg_v", [128, S // 2], f32,
                                       kind="ExternalOutput").ap()
                nc.sync.dma_start(dbg_kt[:], kt_rope[:].bitcast(f32))
                nc.sync.dma_start(dbg_q0[:], qt_rope[0][:].bitcast(f32))
                nc.sync.dma_start(dbg_v[:], v_all[:].bitcast(f32))  # raw bits

            # ---- phase C: ReduceScatter partials, emit this core's slice ----
            if with_rs:
                nc.gpsimd.collective_compute(
                    "ReduceScatter", ADD,
                    replica_groups=[[0, 1, 2, 3], [4, 5, 6, 7]],
                    ins=[oT_part[:].opt()], outs=[oT_red[:].opt()],
                )
                nc.sync.dma_start(out_r[:], oT_red[:])
            # else: emit_oproj already wrote out_r directly

    nc.compile()
    return nc


def kernel(hidden_states, attention_mask, Wq, Wk, Wv, Wo, sin, cos):
    hidden_states = np.asarray(hidden_states, dtype=np.float32)
    attention_mask = np.asarray(attention_mask, dtype=np.float32)
    Wq, Wk, Wv, Wo = (np.ascontiguousarray(np.asarray(a, dtype=np.float32))
                      for a in (Wq, Wk, Wv, Wo))
    sin = np.asarray(sin, dtype=np.float32)
    cos = np.asarray(cos, dtype=np.float32)

    # classify the mask: causal (top-right strictly very-negative, elsewhere 0,
    # col 0 ignored since reference zeroes it) vs all-zeros (full attention)
    m0 = attention_mask[0, 0]
    iu = np.triu_indices(S, k=1)
    causal = bool((m0[iu] < -1e30).all() and
                  (m0[np.tril_indices(S, k=0)] == 0.0).all())
    if not causal:
        assert (attention_mask == 0).all(), "unsupported attention mask pattern"
    if causal:
        for b in range(1, B):
            assert np.array_equal(attention_mask[b, 0], m0), "mask differs per batch"

    key = causal
    if key not in _CACHE:
        _CACHE[key] = _build(causal)
    nc = _CACHE[key]

    cos_t = np.ascontiguousarray(cos[:S].T)          # [128, S]
    sin_t = cos_t.copy()
    sin_t[:] = sin[:S].T
    sin_m = sin_t.copy()
    sin_m[:64] *= -1.0
    # 0/1 causal keep-patterns: patt0 = (q >= k), patt1 = (q >= k + 128)
    kl = np.arange(128)[:, None]
    ql = np.arange(512)[None, :]
    cmask = np.concatenate(
        [(ql >= kl).astype(np.float32),
         (ql[:, :256] >= kl + 128).astype(np.float32)], axis=1)

    in_maps = []
    for c in range(8):
        b, g = c // 4, c % 4
        in_maps.append({
            "xt": np.ascontiguousarray(hidden_states[b].T),
            "wq": np.ascontiguousarray(Wq[512 * g:512 * (g + 1), :].T),
            "wk": np.ascontiguousarray(Wk[128 * g:128 * (g + 1), :].T),
            "wv": np.ascontiguousarray(Wv[128 * g:128 * (g + 1), :].T),
            "wo": np.ascontiguousarray(Wo[:, 512 * g:512 * (g + 1)].T),
            "cos_t": cos_t, "sin_m": sin_m, "cmask": cmask,
            "ones_in": np.ones((128, 128), dtype=np.float32),
            "ident_in": np.eye(128, dtype=np.float32),
        })

    global _LAST_IN_MAPS, _LAST_RES
    _LAST_IN_MAPS = in_maps
    res = run_bass_kernel_spmd(nc, in_maps, core_ids=list(range(8)))
    _LAST_RES = res

    out = np.empty((B, S, HID), dtype=np.float32)
    for c in range(8):
        b, r = c // 4, c % 4
        out[b, :, TB * r:TB * (r + 1)] = res.results[c]["out_r"].T
    return out


if __name__ == "__main__":
    rng = np.random.default_rng(0)
    h = rng.standard_normal((B, S, HID), dtype=np.float32)
    print("module loads ok")


# revision 55
# speedup vs baseline: 1.3691x; 1.0300x over previous
"""HF OpenMoe attention (B=2,S=2048,HID=2048,NH=16,NKV=4,HD=128) on 8 trn2 cores.

Sharding: core c -> (batch b=c//4, kv-group g=c%4). Each core computes Q/K/V
projections for its 4 query heads + 1 kv head, RoPE, causal flash attention in
S^T layout (scores transposed: [k, q], softmax over the partition dim via
ones-matmul), and its partial o_proj; a 4-way ReduceScatter sums the o_proj
partials, each core returning a 512-row slice of o^T for its batch.

Engine balance: PE does only matmuls (f32r for projections/scores/o_proj,
bf16 for P.V), exp runs on the scalar engine (bf16 out), elementwise work is
split between the vector (DVE) and pool (gpsimd) engines, rotate-half swaps
are SBUF DMAs on the pool queue, V transposes use the DMA xbar, and the causal
diagonal is tightened to 512/384/256/256-wide sub-tiles.
"""
import numpy as np
import concourse.bass as bass
import concourse.bacc as bacc
import concourse.tile as tile
import concourse.mybir as mybir
from concourse.bass_utils import run_bass_kernel_spmd

f32 = mybir.dt.float32
f32r = mybir.dt.float32r
bf16 = mybir.dt.bfloat16
AF = mybir.ActivationFunctionType
MUL = mybir.AluOpType.mult
ADD = mybir.AluOpType.add

B, S, HID = 2, 2048, 2048
NH, NKV, HD = 16, 4, 128
GH = NH // NKV          # query heads per core (4)
TB = 512                # token block (q block / projection block)
NT = S // TB            # 4 token blocks
NCT = HID // 128        # 16 contraction tiles
NKT = S // 128          # 16 key tiles

_CACHE = {}
_DEBUG_OUTS = False


def _build(causal: bool, with_rs: bool = True):
    nc = bacc.Bacc("TRN2", target_bir_lowering=False, debug=False, num_devices=8)
    xt = nc.dram_tensor("xt", [HID, S], f32, kind="ExternalInput").ap()
    wq = nc.dram_tensor("wq", [HID, GH * HD], f32, kind="ExternalInput").ap()
    wk = nc.dram_tensor("wk", [HID, HD], f32, kind="ExternalInput").ap()
    wv = nc.dram_tensor("wv", [HID, HD], f32, kind="ExternalInput").ap()
    wo = nc.dram_tensor("wo", [GH * HD, HID], f32, kind="ExternalInput").ap()
    cos_d = nc.dram_tensor("cos_t", [HD, S], f32, kind="ExternalInput").ap()
    sin_d = nc.dram_tensor("sin_m", [HD, S], f32, kind="ExternalInput").ap()
    cm_d = nc.dram_tensor("cmask", [128, 768], f32, kind="ExternalInput").ap()
    on_d = nc.dram_tensor("ones_in", [128, 128], f32, kind="ExternalInput").ap()
    id_d = nc.dram_tensor("ident_in", [128, 128], f32, kind="ExternalInput").ap()
    out_r = nc.dram_tensor("out_r", [TB, S], f32, kind="ExternalOutput").ap()

    with tile.TileContext(nc) as tc:
        with (
            tc.tile_pool(name="glob", bufs=1) as glob,
            tc.tile_pool(name="dram", bufs=1, space="DRAM") as dram,
        ):
            # ---- global resident stores ----
            kt_rope = glob.tile([128, S], f32r, tag="kt")          # roped K^T [d, k]
            v_all = glob.tile([128, S], bf16, tag="v")             # V natural (bf16), 128i:+128 = tile i
            qt_rope = [glob.tile([128, S], f32r, tag=f"q{h}", name=f"qt_rope{h}")
                       for h in range(GH)]
            cm_b = glob.tile([128, 768], bf16, tag="cmb")          # 0/1 keep masks (2 patterns)
            ones_b = glob.tile([128, 128], bf16, tag="onesb")
            ident_b = glob.tile([128, 128], bf16, tag="identb")

            oT_part = dram.tile([HID, S], f32)                     # o^T partial
            oT_red = dram.tile([TB, S], f32)

            # ---- phase A: projections + rope (phase-scoped SBUF) ----
            with tc.tile_pool(name="pA", bufs=1) as pA, \
                 tc.tile_pool(name="psA", bufs=1, space="PSUM") as psA:
                # batched weight loads: DRAM [c, d] -> SBUF [c-sub(128), ci, d].
                # Issue order matters: the DMA engines drain in order, so load
                # what phase A needs first (wk, rope tables), then xt(tb0) is
                # issued inside the loop, then wv/wq, and wo during tb1.
                wk_all = pA.tile([128, NCT, HD], f32r, tag="wk")
                nc.sync.dma_start(wk_all[:, 0:4, :], wk[0:512, :].bitcast(f32r)
                                  .rearrange("(c p) d -> p c d", p=128))
                id_f = pA.tile([128, 128], f32, tag="idf")
                nc.sync.dma_start(id_f[:], id_d[:])
                nc.vector.tensor_copy(ident_b[:], id_f[:])
                warm = pA.tile([128, 1], f32, tag="warm")
                nc.scalar.activation(warm[:], id_f[:, 0:1], AF.Exp)
                wv_all = pA.tile([128, NCT, HD], f32r, tag="wv")
                wq_all = [pA.tile([128, NCT, HD], f32r, tag=f"wqh{h}",
                                  name=f"wq_all{h}") for h in range(GH)]

                def rope(ps, dst_ap, cs, sn, flip):
                    """dst = ps*cos + swap64(ps)*sin_mod for token block tb."""
                    raw = pA.tile([128, TB], f32, tag="raw", bufs=2, name="raw")
                    nc.scalar.copy(raw[:], ps[:])
                    rot = pA.tile([128, TB], f32, tag="rot", bufs=6, name="rot")
                    nc.gpsimd.dma_start(rot[0:64, :], raw[64:128, :])
                    nc.gpsimd.dma_start(rot[64:128, :], raw[0:64, :])
                    m1 = pA.tile([128, TB], f32, tag="m1", bufs=6, name="m1")
                    nc.vector.tensor_tensor(m1[:], ps[:], cs[:], op=MUL)  # PSUM: DVE
                    nc.vector.tensor_tensor(rot[:], rot[:], sn[:], op=MUL)
                    nc.vector.tensor_tensor(dst_ap, m1[:], rot[:], op=ADD)

                for tb in range(NT):
                    cos_s = pA.tile([128, TB], f32, tag="cos", bufs=4, name="cos")
                    sin_s = pA.tile([128, TB], f32, tag="sin", bufs=4, name="sin")
                    if tb != 0:
                        # rope tables just ahead of the xt tiles
                        nc.sync.dma_start(cos_s[:], cos_d[:, TB * tb:TB * (tb + 1)])
                        nc.sync.dma_start(sin_s[:], sin_d[:, TB * tb:TB * (tb + 1)])
                    xt_t = []
                    for ci in range(NCT):
                        t = pA.tile([128, TB], f32r, tag="xt", bufs=32, name="xt")
                        nc.sync.dma_start(
                            t[:], xt[128 * ci:128 * (ci + 1),
                                     TB * tb:TB * (tb + 1)].bitcast(f32r))
                        xt_t.append(t)
                        if tb == 0 and ci % 4 == 3 and ci < 15:
                            c = ci // 4 + 1  # stream wk in behind the xt tiles
                            nc.sync.dma_start(
                                wk_all[:, 4 * c:4 * (c + 1), :],
                                wk[512 * c:512 * (c + 1), :].bitcast(f32r)
                                .rearrange("(c p) d -> p c d", p=128))
                        if tb == 0 and ci == 0:
                            nc.sync.dma_start(cos_s[:], cos_d[:, 0:TB])
                            nc.sync.dma_start(sin_s[:], sin_d[:, 0:TB])
                        if tb == 0 and ci == 10:
                            # wv lands right as the V matmuls want it; K's last
                            # xt tiles shift later but K ends DMA-paced anyway
                            nc.sync.dma_start(wv_all[:], wv[:].bitcast(f32r)
                                              .rearrange("(c p) d -> p c d", p=128))
                    if tb == 0:
                        for h in range(GH):
                            nc.sync.dma_start(
                                wq_all[h][:], wq[:, HD * h:HD * (h + 1)]
                                .bitcast(f32r).rearrange("(c p) d -> p c d", p=128))
                    # K
                    ps_k = psA.tile([128, TB], f32, tag="pk")
                    for ci in range(NCT):
                        nc.tensor.matmul(ps_k[:], wk_all[:, ci, :], xt_t[ci][:],
                                         start=ci == 0, stop=ci == NCT - 1)
                    rope(ps_k, kt_rope[:, TB * tb:TB * (tb + 1)], cos_s, sin_s,
                         flip=True)
                    # V
                    ps_v = psA.tile([128, TB], f32, tag="pv")
                    for ci in range(NCT):
                        nc.tensor.matmul(ps_v[:], wv_all[:, ci, :], xt_t[ci][:],
                                         start=ci == 0, stop=ci == NCT - 1)
                    vt_sb = pA.tile([128, TB], bf16, tag="vts", bufs=2, name="vt_sb")
                    nc.scalar.copy(vt_sb[:], ps_v[:])
                    # Q heads; one V transpose is spread between each pair of
                    # head blocks so the single ptr bank's WAR (on the previous
                    # transpose's drain copy) never stalls the PE
                    for h in range(GH):
                        # ps_q: one shared 5-deep rotation across all heads and
                        # blocks -- the slot reused by Q_h(tb+1) was freed more
                        # than a full block ago, so the 3-engine rope chain of
                        # block tb never back-pressures the next block
                        ps_q = psA.tile([128, TB], f32, tag="pq", bufs=5,
                                        name=f"ps_q{h}")
                        for ci in range(NCT):
                            nc.tensor.matmul(ps_q[:], wq_all[h][:, ci, :], xt_t[ci][:],
                                             start=ci == 0, stop=ci == NCT - 1)
                        u = h
                        ps_tr = psA.tile([128, 128], bf16, tag="ptr", bufs=1,
                                         name="ps_tr")
                        nc.tensor.transpose(ps_tr[:], vt_sb[:, 128 * u:128 * (u + 1)],
                                            ident_b[:])
                        # Act (not DVE): DVE's in-order queue sits behind rope
                        # m1 ops that can wait on the cos/sin loads
                        nc.scalar.copy(
                            v_all[:, 128 * (4 * tb + u):128 * (4 * tb + u + 1)],
                            ps_tr[:])
                        rope(ps_q, qt_rope[h][:, TB * tb:TB * (tb + 1)], cos_s,
                             sin_s, flip=(h % 2 == 0))

            # ---- phase B: attention + partial o_proj (phase-scoped SBUF) ----
            with tc.tile_pool(name="pB", bufs=1) as pB, \
                 tc.tile_pool(name="psB", bufs=1, space="PSUM") as psB:
                # one-time small loads + casts (tiles scoped here, used here)
                cm_f = pB.tile([128, 768], f32, tag="cmf")
                nc.sync.dma_start(cm_f[:], cm_d[:])
                nc.vector.tensor_copy(cm_b[:], cm_f[:])
                on_f = pB.tile([128, 128], f32, tag="onf")
                nc.sync.dma_start(on_f[:], on_d[:])
                nc.vector.tensor_copy(ones_b[:], on_f[:])
                # o_proj weights: first o_proj runs ~25us into phase B, so this
                # load hides behind the first attention block
                wo_all = pB.tile([128, GH * HID], f32r, tag="wo")  # [j-sub, jh*2048+c]
                nc.sync.dma_start(wo_all[:].rearrange("p (h c) -> p h c", h=GH),
                                  wo[:].bitcast(f32r)
                                  .rearrange("(h p) c -> p h c", p=128))

                def oproj_chunk(j, at_j, cb):
                    """One 4-co-tile chunk of q-block j's o_proj, batched into a
                    single output DMA."""
                    ob4 = pB.tile([128, 4, TB], f32, tag="ob", bufs=2,
                                  name="ob4")
                    for u in range(4):
                        co = 4 * cb + u
                        ps_p = psB.tile([128, TB], f32, tag="ps_d", bufs=2,
                                        name="ps_p")
                        for jh in range(GH):
                            nc.tensor.matmul(ps_p[:],
                                             wo_all[:, jh * HID + 128 * co:
                                                    jh * HID + 128 * (co + 1)],
                                             at_j[jh][:], start=(jh == 0),
                                             stop=(jh == GH - 1))
                        if u % 2 == 1:
                            nc.scalar.copy(ob4[:, u, :], ps_p[:])
                        else:
                            nc.vector.tensor_copy(ob4[:, u, :], ps_p[:])
                    if cb == 0 and not with_rs:
                        # no-collective mode returns oT_part[0:512] as out_r;
                        # write it there directly
                        dst = out_r[:, TB * j:TB * (j + 1)]
                    else:
                        dst = oT_part[512 * cb:512 * (cb + 1),
                                      TB * j:TB * (j + 1)]
                    nc.sync.dma_start(
                        dst.rearrange("(u p) t -> p u t", p=128), ob4[:])

                pending = []  # deferred o_proj chunk closures
                # q-blocks in rotated order so the final (tail) o_proj pairs
                # with the SHORT j=0 block: its at_s chain hides behind the
                # previous block's o_proj instead of stalling the PE
                for j in ([1, 2, 3, 0] if causal else range(NT)):
                    # (i, q-col-offset) tile list: full tiles then tightened diagonal
                    if causal:
                        tiles = [(i, 0) for i in range(4 * j)]
                        tiles += [(4 * j + m, min(128 * m, 256)) for m in range(4)]
                    else:
                        tiles = [(i, 0) for i in range(NKT)]
                    last_i = tiles[-1][0]

                    ps_o = [psB.tile([128, TB], f32, tag="po", bufs=4, name=f"ps_o{h}")
                            for h in range(GH)]
                    acc = [pB.tile([128, TB], bf16, tag=f"acc{h}", bufs=1,
                                   name=f"acch{h}") for h in range(GH)]
                    av_pend = []  # (h, i, off, w, pt) AVs deferred from i<2

                    def flush_av():
                        for (fh, fi, foff, fw, fpt) in av_pend:
                            nc.tensor.matmul(ps_o[fh][:, foff:TB],
                                             v_all[:, 128 * fi:128 * (fi + 1)],
                                             fpt[:, 0:fw],
                                             start=(fi == 0), stop=(fi == last_i),
                                             skip_group_check=True)
                        av_pend.clear()

                    for ti, (i, off) in enumerate(tiles):
                        w = TB - off
                        diag = causal and i >= 4 * j
                        m = i - 4 * j if diag else -1
                        if i == 2:
                            # the i<2 AVs were deferred so the first PSUM write
                            # to the po banks (WAR on last block's at_s) comes
                            # after ~8 scores' worth of PE work
                            flush_av()
                        for h in range(GH):
                            ps_s = psB.tile([128, TB], f32, tag="ps_s", bufs=2,
                                            name="ps_s")
                            nc.tensor.matmul(ps_s[:, 0:w],
                                             kt_rope[:, 128 * i:128 * (i + 1)],
                                             qt_rope[h][:, TB * j + off:TB * (j + 1)],
                                             start=True, stop=True)
                            pt = pB.tile([128, TB], bf16, tag="pt", bufs=8, name="pt")
                            nc.scalar.activation(pt[:, 0:w], ps_s[:, 0:w], AF.Exp)
                            if diag:
                                patt = 512 if m == 3 else 0
                                meng = nc.gpsimd if w <= 256 else nc.vector
                                meng.tensor_tensor(
                                    pt[:, 0:w], pt[:, 0:w],
                                    cm_b[:, patt:patt + w], op=MUL)
                            if i == 0:
                                nc.gpsimd.tensor_copy(acc[h][:], pt[:])
                            else:
                                nc.vector.tensor_tensor(acc[h][:, off:TB],
                                                        acc[h][:, off:TB],
                                                        pt[:, 0:w], op=ADD)
                            if i < 2:
                                av_pend.append((h, i, off, w, pt))
                                continue
                            nc.tensor.matmul(ps_o[h][:, off:TB],
                                             v_all[:, 128 * i:128 * (i + 1)],
                                             pt[:, 0:w],
                                             start=False, stop=(i == last_i),
                                             skip_group_check=True)
                    flush_av()
                    # any previous-block o_proj chunks not yet interleaved
                    while pending:
                        pending.pop(0)()

                    # normalize into A^T blocks
                    at_s = [pB.tile([128, TB], f32r, tag=f"at{h}", bufs=1,
                                    name=f"at_s{h}") for h in range(GH)]
                    for h in range(GH):
                        ps_d = psB.tile([128, TB], f32, tag="ps_d", bufs=2,
                                        name="ps_d")
                        nc.tensor.matmul(ps_d[:], ones_b[:], acc[h][:],
                                         start=True, stop=True)
                        rec = pB.tile([128, TB], f32, tag="rec", bufs=2, name="rec")
                        nc.vector.reciprocal(rec[:], ps_d[:])
                        nc.vector.tensor_tensor(at_s[h][:], ps_o[h][:], rec[:],
                                                op=MUL)  # PSUM: DVE only
                    pending = [
                        (lambda jj, aa, cc: lambda: oproj_chunk(jj, aa, cc))(
                            j, at_s, cb) for cb in range(NCT // 4)]
                while pending:
                    pending.pop(0)()

            if _DEBUG_OUTS:
                dbg_kt = nc.dram_tensor("dbg_kt", [128, S], f32,
                                        kind="ExternalOutput").ap()
                dbg_q0 = nc.dram_tensor("dbg_q0", [128, S], f32,
                                        kind="ExternalOutput").ap()
                dbg_v = nc.dram_tensor("dbg_v", [128, S // 2], f32,
                                       kind="ExternalOutput").ap()
                nc.sync.dma_start(dbg_kt[:], kt_rope[:].bitcast(f32))
                nc.sync.dma_start(dbg_q0[:], qt_rope[0][:].bitcast(f32))
                nc.sync.dma_start(dbg_v[:], v_all[:].bitcast(f32))  # raw bits

            # ---- phase C: ReduceScatter partials, emit this core's slice ----
            if with_rs:
                nc.gpsimd.collective_compute(
                    "ReduceScatter", ADD,
                    replica_groups=[[0, 1, 2, 3], [4, 5, 6, 7]],
                    ins=[oT_part[:].opt()], outs=[oT_red[:].opt()],
                )
                nc.sync.dma_start(out_r[:], oT_red[:])
            # else: emit_oproj already wrote out_r directly

    nc.compile()
    return nc


def kernel(hidden_states, attention_mask, Wq, Wk, Wv, Wo, sin, cos):
    hidden_states = np.asarray(hidden_states, dtype=np.float32)
    attention_mask = np.asarray(attention_mask, dtype=np.float32)
    Wq, Wk, Wv, Wo = (np.ascontiguousarray(np.asarray(a, dtype=np.float32))
                      for a in (Wq, Wk, Wv, Wo))
    sin = np.asarray(sin, dtype=np.float32)
    cos = np.asarray(cos, dtype=np.float32)

    # classify the mask: causal (top-right strictly very-negative, elsewhere 0,
    # col 0 ignored since reference zeroes it) vs all-zeros (full attention)
    m0 = attention_mask[0, 0]
    iu = np.triu_indices(S, k=1)
    causal = bool((m0[iu] < -1e30).all() and
                  (m0[np.tril_indices(S, k=0)] == 0.0).all())
    if not causal:
        assert (attention_mask == 0).all(), "unsupported attention mask pattern"
    if causal:
        for b in range(1, B):
            assert np.array_equal(attention_mask[b, 0], m0), "mask differs per batch"

    key = causal
    if key not in _CACHE:
        _CACHE[key] = _build(causal)
    nc = _CACHE[key]

    cos_t = np.ascontiguousarray(cos[:S].T)          # [128, S]
    sin_t = cos_t.copy()
    sin_t[:] = sin[:S].T
    sin_m = sin_t.copy()
    sin_m[:64] *= -1.0
    # 0/1 causal keep-patterns: patt0 = (q >= k), patt1 = (q >= k + 128)
    kl = np.arange(128)[:, None]
    ql = np.arange(512)[None, :]
    cmask = np.concatenate(
        [(ql >= kl).astype(np.float32),
         (ql[:, :256] >= kl + 128).astype(np.float32)], axis=1)

    in_maps = []
    for c in range(8):
        b, g = c // 4, c % 4
        in_maps.append({
            "xt": np.ascontiguousarray(hidden_states[b].T),
            "wq": np.ascontiguousarray(Wq[512 * g:512 * (g + 1), :].T),
            "wk": np.ascontiguousarray(Wk[128 * g:128 * (g + 1), :].T),
            "wv": np.ascontiguousarray(Wv[128 * g:128 * (g + 1), :].T),
            "wo": np.ascontiguousarray(Wo[:, 512 * g:512 * (g + 1)].T),
            "cos_t": cos_t, "sin_m": sin_m, "cmask": cmask,
            "ones_in": np.ones((128, 128), dtype=np.float32),
            "ident_in": np.eye(128, dtype=np.float32),
        })

    global _LAST_IN_MAPS, _LAST_RES
    _LAST_IN_MAPS = in_maps
    res = run_bass_kernel_spmd(nc, in_maps, core_ids=list(range(8)))
    _LAST_RES = res

    out = np.empty((B, S, HID), dtype=np.float32)
    for c in range(8):
        b, r = c // 4, c % 4
        out[b, :, TB * r:TB * (r + 1)] = res.results[c]["out_r"].T
    return out


if __name__ == "__main__":
    rng = np.random.default_rng(0)
    h = rng.standard_normal((B, S, HID), dtype=np.float32)
    print("module loads ok")


# revision 56
# speedup vs baseline: 1.3931x; 1.0175x over previous
"""HF OpenMoe attention (B=2,S=2048,HID=2048,NH=16,NKV=4,HD=128) on 8 trn2 cores.

Sharding: core c -> (batch b=c//4, kv-group g=c%4). Each core computes Q/K/V
projections for its 4 query heads + 1 kv head, RoPE, causal flash attention in
S^T layout (scores transposed: [k, q], softmax over the partition dim via
ones-matmul), and its partial o_proj; a 4-way ReduceScatter sums the o_proj
partials, each core returning a 512-row slice of o^T for its batch.

Engine balance: PE does only matmuls (f32r for projections/scores/o_proj,
bf16 for P.V), exp runs on the scalar engine (bf16 out), elementwise work is
split between the vector (DVE) and pool (gpsimd) engines, rotate-half swaps
are SBUF DMAs on the pool queue, V transposes use the DMA xbar, and the causal
diagonal is tightened to 512/384/256/256-wide sub-tiles.
"""
import numpy as np
import concourse.bass as bass
import concourse.bacc as bacc
import concourse.tile as tile
import concourse.mybir as mybir
from concourse.bass_utils import run_bass_kernel_spmd

f32 = mybir.dt.float32
f32r = mybir.dt.float32r
bf16 = mybir.dt.bfloat16
AF = mybir.ActivationFunctionType
MUL = mybir.AluOpType.mult
ADD = mybir.AluOpType.add

B, S, HID = 2, 2048, 2048
NH, NKV, HD = 16, 4, 128
GH = NH // NKV          # query heads per core (4)
TB = 512                # token block (q block / projection block)
NT = S // TB            # 4 token blocks
NCT = HID // 128        # 16 contraction tiles
NKT = S // 128          # 16 key tiles

_CACHE = {}
_DEBUG_OUTS = False


def _build(causal: bool, with_rs: bool = True):
    nc = bacc.Bacc("TRN2", target_bir_lowering=False, debug=False, num_devices=8)
    xt = nc.dram_tensor("xt", [HID, S], f32, kind="ExternalInput").ap()
    wq = nc.dram_tensor("wq", [HID, GH * HD], f32, kind="ExternalInput").ap()
    wk = nc.dram_tensor("wk", [HID, HD], f32, kind="ExternalInput").ap()
    wv = nc.dram_tensor("wv", [HID, HD], f32, kind="ExternalInput").ap()
    wo = nc.dram_tensor("wo", [GH * HD, HID], f32, kind="ExternalInput").ap()
    cos_d = nc.dram_tensor("cos_t", [HD, S], f32, kind="ExternalInput").ap()
    sin_d = nc.dram_tensor("sin_m", [HD, S], f32, kind="ExternalInput").ap()
    cm_d = nc.dram_tensor("cmask", [128, 768], f32, kind="ExternalInput").ap()
    on_d = nc.dram_tensor("ones_in", [128, 128], f32, kind="ExternalInput").ap()
    id_d = nc.dram_tensor("ident_in", [128, 128], f32, kind="ExternalInput").ap()
    out_r = nc.dram_tensor("out_r", [TB, S], f32, kind="ExternalOutput").ap()

    with tile.TileContext(nc) as tc:
        with (
            tc.tile_pool(name="glob", bufs=1) as glob,
            tc.tile_pool(name="dram", bufs=1, space="DRAM") as dram,
        ):
            # ---- global resident stores ----
            kt_rope = glob.tile([128, S], f32r, tag="kt")          # roped K^T [d, k]
            v_all = glob.tile([128, S], bf16, tag="v")             # V natural (bf16), 128i:+128 = tile i
            qt_rope = [glob.tile([128, S], f32r, tag=f"q{h}", name=f"qt_rope{h}")
                       for h in range(GH)]
            cm_b = glob.tile([128, 768], bf16, tag="cmb")          # 0/1 keep masks (2 patterns)
            ones_b = glob.tile([128, 128], bf16, tag="onesb")
            ident_b = glob.tile([128, 128], bf16, tag="identb")

            oT_part = dram.tile([HID, S], f32)                     # o^T partial
            oT_red = dram.tile([TB, S], f32)

            # ---- phase A: projections + rope (phase-scoped SBUF) ----
            with tc.tile_pool(name="pA", bufs=1) as pA, \
                 tc.tile_pool(name="psA", bufs=1, space="PSUM") as psA:
                # batched weight loads: DRAM [c, d] -> SBUF [c-sub(128), ci, d].
                # Issue order matters: the DMA engines drain in order, so load
                # what phase A needs first (wk, rope tables), then xt(tb0) is
                # issued inside the loop, then wv/wq, and wo during tb1.
                wk_all = pA.tile([128, NCT, HD], f32r, tag="wk")
                nc.sync.dma_start(wk_all[:, 0:4, :], wk[0:512, :].bitcast(f32r)
                                  .rearrange("(c p) d -> p c d", p=128))
                id_f = pA.tile([128, 128], f32, tag="idf")
                nc.sync.dma_start(id_f[:], id_d[:])
                nc.vector.tensor_copy(ident_b[:], id_f[:])
                warm = pA.tile([128, 1], f32, tag="warm")
                nc.scalar.activation(warm[:], id_f[:, 0:1], AF.Exp)
                wv_all = pA.tile([128, NCT, HD], f32r, tag="wv")
                wq_all = [pA.tile([128, NCT, HD], f32r, tag=f"wqh{h}",
                                  name=f"wq_all{h}") for h in range(GH)]

                def rope(ps, dst_ap, cs, sn, flip):
                    """dst = ps*cos + swap64(ps)*sin_mod for token block tb."""
                    raw = pA.tile([128, TB], f32, tag="raw", bufs=2, name="raw")
                    nc.scalar.copy(raw[:], ps[:])
                    rot = pA.tile([128, TB], f32, tag="rot", bufs=6, name="rot")
                    nc.gpsimd.dma_start(rot[0:64, :], raw[64:128, :])
                    nc.gpsimd.dma_start(rot[64:128, :], raw[0:64, :])
                    m1 = pA.tile([128, TB], f32, tag="m1", bufs=6, name="m1")
                    nc.vector.tensor_tensor(m1[:], ps[:], cs[:], op=MUL)  # PSUM: DVE
                    nc.vector.tensor_tensor(rot[:], rot[:], sn[:], op=MUL)
                    nc.vector.tensor_tensor(dst_ap, m1[:], rot[:], op=ADD)

                for tb in range(NT):
                    cos_s = pA.tile([128, TB], f32, tag="cos", bufs=4, name="cos")
                    sin_s = pA.tile([128, TB], f32, tag="sin", bufs=4, name="sin")
                    if tb != 0:
                        # rope tables just ahead of the xt tiles
                        nc.sync.dma_start(cos_s[:], cos_d[:, TB * tb:TB * (tb + 1)])
                        nc.sync.dma_start(sin_s[:], sin_d[:, TB * tb:TB * (tb + 1)])
                    xt_t = []
                    for ci in range(NCT):
                        t = pA.tile([128, TB], f32r, tag="xt", bufs=32, name="xt")
                        nc.sync.dma_start(
                            t[:], xt[128 * ci:128 * (ci + 1),
                                     TB * tb:TB * (tb + 1)].bitcast(f32r))
                        xt_t.append(t)
                        if tb == 0 and ci % 4 == 3 and ci < 15:
                            c = ci // 4 + 1  # stream wk in behind the xt tiles
                            nc.sync.dma_start(
                                wk_all[:, 4 * c:4 * (c + 1), :],
                                wk[512 * c:512 * (c + 1), :].bitcast(f32r)
                                .rearrange("(c p) d -> p c d", p=128))
                        if tb == 0 and ci == 0:
                            nc.sync.dma_start(cos_s[:], cos_d[:, 0:TB])
                            nc.sync.dma_start(sin_s[:], sin_d[:, 0:TB])
                        if tb == 0 and ci == 10:
                            # wv lands right as the V matmuls want it; K's last
                            # xt tiles shift later but K ends DMA-paced anyway
                            nc.sync.dma_start(wv_all[:], wv[:].bitcast(f32r)
                                              .rearrange("(c p) d -> p c d", p=128))
                    if tb == 0:
                        for h in range(GH):
                            nc.sync.dma_start(
                                wq_all[h][:], wq[:, HD * h:HD * (h + 1)]
                                .bitcast(f32r).rearrange("(c p) d -> p c d", p=128))
                    # K
                    ps_k = psA.tile([128, TB], f32, tag="pk")
                    for ci in range(NCT):
                        nc.tensor.matmul(ps_k[:], wk_all[:, ci, :], xt_t[ci][:],
                                         start=ci == 0, stop=ci == NCT - 1)
                    rope(ps_k, kt_rope[:, TB * tb:TB * (tb + 1)], cos_s, sin_s,
                         flip=True)
                    # V
                    ps_v = psA.tile([128, TB], f32, tag="pv")
                    for ci in range(NCT):
                        nc.tensor.matmul(ps_v[:], wv_all[:, ci, :], xt_t[ci][:],
                                         start=ci == 0, stop=ci == NCT - 1)
                    vt_sb = pA.tile([128, TB], bf16, tag="vts", bufs=2, name="vt_sb")
                    nc.scalar.copy(vt_sb[:], ps_v[:])
                    # Q heads; one V transpose is spread between each pair of
                    # head blocks so the single ptr bank's WAR (on the previous
                    # transpose's drain copy) never stalls the PE
                    for h in range(GH):
                        # ps_q: one shared 5-deep rotation across all heads and
                        # blocks -- the slot reused by Q_h(tb+1) was freed more
                        # than a full block ago, so the 3-engine rope chain of
                        # block tb never back-pressures the next block
                        ps_q = psA.tile([128, TB], f32, tag="pq", bufs=5,
                                        name=f"ps_q{h}")
                        for ci in range(NCT):
                            nc.tensor.matmul(ps_q[:], wq_all[h][:, ci, :], xt_t[ci][:],
                                             start=ci == 0, stop=ci == NCT - 1)
                        u = h
                        ps_tr = psA.tile([128, 128], bf16, tag="ptr", bufs=1,
                                         name="ps_tr")
                        nc.tensor.transpose(ps_tr[:], vt_sb[:, 128 * u:128 * (u + 1)],
                                            ident_b[:])
                        # Act (not DVE): DVE's in-order queue sits behind rope
                        # m1 ops that can wait on the cos/sin loads
                        nc.scalar.copy(
                            v_all[:, 128 * (4 * tb + u):128 * (4 * tb + u + 1)],
                            ps_tr[:])
                        rope(ps_q, qt_rope[h][:, TB * tb:TB * (tb + 1)], cos_s,
                             sin_s, flip=(h % 2 == 0))

            # ---- phase B: attention + partial o_proj (phase-scoped SBUF) ----
            with tc.tile_pool(name="pB", bufs=1) as pB, \
                 tc.tile_pool(name="psB", bufs=1, space="PSUM") as psB:
                # one-time small loads + casts (tiles scoped here, used here)
                cm_f = pB.tile([128, 768], f32, tag="cmf")
                nc.sync.dma_start(cm_f[:], cm_d[:])
                nc.vector.tensor_copy(cm_b[:], cm_f[:])
                on_f = pB.tile([128, 128], f32, tag="onf")
                nc.sync.dma_start(on_f[:], on_d[:])
                nc.vector.tensor_copy(ones_b[:], on_f[:])
                # o_proj weights: first o_proj runs ~25us into phase B, so this
                # load hides behind the first attention block
                wo_all = pB.tile([128, GH * HID], f32r, tag="wo")  # [j-sub, jh*2048+c]
                nc.sync.dma_start(wo_all[:].rearrange("p (h c) -> p h c", h=GH),
                                  wo[:].bitcast(f32r)
                                  .rearrange("(h p) c -> p h c", p=128))

                def oproj_chunk(j, at_j, cb, last=False):
                    """One 4-co-tile chunk of q-block j's o_proj, batched into a
                    single output DMA."""
                    ob4 = pB.tile([128, 4, TB], f32, tag="ob", bufs=3,
                                  name="ob4")
                    for u in range(4):
                        co = 4 * cb + u
                        ps_p = psB.tile([128, TB], f32, tag="ps_d", bufs=2,
                                        name="ps_p")
                        for jh in range(GH):
                            nc.tensor.matmul(ps_p[:],
                                             wo_all[:, jh * HID + 128 * co:
                                                    jh * HID + 128 * (co + 1)],
                                             at_j[jh][:], start=(jh == 0),
                                             stop=(jh == GH - 1))
                        if u % 2 == 1:
                            nc.scalar.copy(ob4[:, u, :], ps_p[:])
                        else:
                            nc.vector.tensor_copy(ob4[:, u, :], ps_p[:])
                    if cb == 0 and not with_rs:
                        # no-collective mode returns oT_part[0:512] as out_r;
                        # write it there directly
                        dst = out_r[:, TB * j:TB * (j + 1)]
                    else:
                        dst = oT_part[512 * cb:512 * (cb + 1),
                                      TB * j:TB * (j + 1)]
                    if last:
                        # two half DMAs so the final transfer tail is shorter
                        d3 = dst.rearrange("(u p) t -> p u t", p=128)
                        nc.sync.dma_start(d3[:, 0:2, :], ob4[:, 0:2, :])
                        nc.sync.dma_start(d3[:, 2:4, :], ob4[:, 2:4, :])
                    else:
                        nc.sync.dma_start(
                            dst.rearrange("(u p) t -> p u t", p=128), ob4[:])

                pending = []  # deferred o_proj chunk closures
                # q-blocks in rotated order so the final (tail) o_proj pairs
                # with the SHORT j=0 block: its at_s chain hides behind the
                # previous block's o_proj instead of stalling the PE
                for j in ([1, 2, 3, 0] if causal else range(NT)):
                    # (i, q-col-offset) tile list: full tiles then tightened diagonal
                    if causal:
                        tiles = [(i, 0) for i in range(4 * j)]
                        tiles += [(4 * j + m, min(128 * m, 256)) for m in range(4)]
                    else:
                        tiles = [(i, 0) for i in range(NKT)]
                    last_i = tiles[-1][0]

                    ps_o = [psB.tile([128, TB], f32, tag="po", bufs=4, name=f"ps_o{h}")
                            for h in range(GH)]
                    acc = [pB.tile([128, TB], bf16, tag=f"acc{h}", bufs=1,
                                   name=f"acch{h}") for h in range(GH)]
                    av_pend = []  # (h, i, off, w, pt) AVs deferred from i<2

                    def flush_av():
                        for (fh, fi, foff, fw, fpt) in av_pend:
                            nc.tensor.matmul(ps_o[fh][:, foff:TB],
                                             v_all[:, 128 * fi:128 * (fi + 1)],
                                             fpt[:, 0:fw],
                                             start=(fi == 0), stop=(fi == last_i),
                                             skip_group_check=True)
                        av_pend.clear()

                    for ti, (i, off) in enumerate(tiles):
                        w = TB - off
                        diag = causal and i >= 4 * j
                        m = i - 4 * j if diag else -1
                        if i == 3:
                            # the i<3 AVs were deferred so the first PSUM write
                            # to the po banks (WAR on last block's at_s) comes
                            # after ~12 scores' worth of PE work
                            flush_av()
                        for h in range(GH):
                            ps_s = psB.tile([128, TB], f32, tag="ps_s", bufs=2,
                                            name="ps_s")
                            nc.tensor.matmul(ps_s[:, 0:w],
                                             kt_rope[:, 128 * i:128 * (i + 1)],
                                             qt_rope[h][:, TB * j + off:TB * (j + 1)],
                                             start=True, stop=True)
                            pt = pB.tile([128, TB], bf16, tag="pt", bufs=16, name="pt")
                            nc.scalar.activation(pt[:, 0:w], ps_s[:, 0:w], AF.Exp)
                            if diag:
                                patt = 512 if m == 3 else 0
                                meng = nc.gpsimd if w <= 256 else nc.vector
                                meng.tensor_tensor(
                                    pt[:, 0:w], pt[:, 0:w],
                                    cm_b[:, patt:patt + w], op=MUL)
                            if i == 0:
                                nc.gpsimd.tensor_copy(acc[h][:], pt[:])
                            else:
                                nc.vector.tensor_tensor(acc[h][:, off:TB],
                                                        acc[h][:, off:TB],
                                                        pt[:, 0:w], op=ADD)
                            if i < 3:
                                av_pend.append((h, i, off, w, pt))
                                continue
                            nc.tensor.matmul(ps_o[h][:, off:TB],
                                             v_all[:, 128 * i:128 * (i + 1)],
                                             pt[:, 0:w],
                                             start=False, stop=(i == last_i),
                                             skip_group_check=True)
                    flush_av()
                    # any previous-block o_proj chunks not yet interleaved
                    while pending:
                        pending.pop(0)()

                    # normalize into A^T blocks
                    at_s = [pB.tile([128, TB], f32r, tag=f"at{h}", bufs=1,
                                    name=f"at_s{h}") for h in range(GH)]
                    for h in range(GH):
                        ps_d = psB.tile([128, TB], f32, tag="ps_d", bufs=2,
                                        name="ps_d")
                        nc.tensor.matmul(ps_d[:], ones_b[:], acc[h][:],
                                         start=True, stop=True)
                        rec = pB.tile([128, TB], f32, tag="rec", bufs=2, name="rec")
                        nc.vector.reciprocal(rec[:], ps_d[:])
                        nc.vector.tensor_tensor(at_s[h][:], ps_o[h][:], rec[:],
                                                op=MUL)  # PSUM: DVE only
                    pending = [
                        (lambda jj, aa, cc: lambda la=False: oproj_chunk(
                            jj, aa, cc, la))(j, at_s, cb)
                        for cb in range(NCT // 4)]
                while len(pending) > 1:
                    pending.pop(0)()
                pending.pop(0)(True)

            if _DEBUG_OUTS:
                dbg_kt = nc.dram_tensor("dbg_kt", [128, S], f32,
                                        kind="ExternalOutput").ap()
                dbg_q0 = nc.dram_tensor("dbg_q0", [128, S], f32,
                                        kind="ExternalOutput").ap()
                dbg_v = nc.dram_tensor("dbg_v", [128, S // 2], f32,
                                       kind="ExternalOutput").ap()
                nc.sync.dma_start(dbg_kt[:], kt_rope[:].bitcast(f32))
                nc.sync.dma_start(dbg_q0[:], qt_rope[0][:].bitcast(f32))
                nc.sync.dma_start(dbg_v[:], v_all[:].bitcast(f32))  # raw bits

            # ---- phase C: ReduceScatter partials, emit this core's slice ----
            if with_rs:
                nc.gpsimd.collective_compute(
                    "ReduceScatter", ADD,
                    replica_groups=[[0, 1, 2, 3], [4, 5, 6, 7]],
                    ins=[oT_part[:].opt()], outs=[oT_red[:].opt()],
                )
                nc.sync.dma_start(out_r[:], oT_red[:])
            # else: emit_oproj already wrote out_r directly

    nc.compile()
    return nc


def kernel(hidden_states, attention_mask, Wq, Wk, Wv, Wo, sin, cos):
    hidden_states = np.asarray(hidden_states, dtype=np.float32)
    attention_mask = np.asarray(attention_mask, dtype=np.float32)
    Wq, Wk, Wv, Wo = (np.ascontiguousarray(np.asarray(a, dtype=np.float32))
                      for a in (Wq, Wk, Wv, Wo))
    sin = np.asarray(sin, dtype=np.float32)
    cos = np.asarray(cos, dtype=np.float32)

    # classify the mask: causal (top-right strictly very-negative, elsewhere 0,
    # col 0 ignored since reference zeroes it) vs all-zeros (full attention)
    m0 = attention_mask[0, 0]
    iu = np.triu_indices(S, k=1)
    causal = bool((m0[iu] < -1e30).all() and
                  (m0[np.tril_indices(S, k=0)] == 0.0).all())
    if not causal:
        assert (attention_mask == 0).all(), "unsupported attention mask pattern"
    if causal:
        for b in range(1, B):
            assert np.array_equal(attention_mask[b, 0], m0), "mask differs per batch"

    key = causal
    if key not in _CACHE:
        _CACHE[key] = _build(causal)
    nc = _CACHE[key]

    cos_t = np.ascontiguousarray(cos[:S].T)          # [128, S]
    sin_t = cos_t.copy()
    sin_t[:] = sin[:S].T
    sin_m = sin_t.copy()
    sin_m[:64] *= -1.0
    # 0/1 causal keep-patterns: patt0 = (q >= k), patt1 = (q >= k + 128)
    kl = np.arange(128)[:, None]
    ql = np.arange(512)[None, :]
    cmask = np.concatenate(
        [(ql >= kl).astype(np.float32),
         (ql[:, :256] >= kl + 128).astype(np.float32)], axis=1)

    in_maps = []
    for c in range(8):
        b, g = c // 4, c % 4
        in_maps.append({
            "xt": np.ascontiguousarray(hidden_states[b].T),
            "wq": np.ascontiguousarray(Wq[512 * g:512 * (g + 1), :].T),
            "wk": np.ascontiguousarray(Wk[128 * g:128 * (g + 1), :].T),
            "wv": np.ascontiguousarray(Wv[128 * g:128 * (g + 1), :].T),
            "wo": np.ascontiguousarray(Wo[:, 512 * g:512 * (g + 1)].T),
            "cos_t": cos_t, "sin_m": sin_m, "cmask": cmask,
            "ones_in": np.ones((128, 128), dtype=np.float32),
            "ident_in": np.eye(128, dtype=np.float32),
        })

    global _LAST_IN_MAPS, _LAST_RES
    _LAST_IN_MAPS = in_maps
    res = run_bass_kernel_spmd(nc, in_maps, core_ids=list(range(8)))
    _LAST_RES = res

    out = np.empty((B, S, HID), dtype=np.float32)
    for c in range(8):
        b, r = c // 4, c % 4
        out[b, :, TB * r:TB * (r + 1)] = res.results[c]["out_r"].T
    return out


if __name__ == "__main__":
    rng = np.random.default_rng(0)
    h = rng.standard_normal((B, S, HID), dtype=np.float32)
    print("module loads ok")


# revision 57
# speedup vs baseline: 1.4105x; 1.0125x over previous
"""HF OpenMoe attention (B=2,S=2048,HID=2048,NH=16,NKV=4,HD=128) on 8 trn2 cores.

Sharding: core c -> (batch b=c//4, kv-group g=c%4). Each core computes Q/K/V
projections for its 4 query heads + 1 kv head, RoPE, causal flash attention in
S^T layout (scores transposed: [k, q], softmax over the partition dim via
ones-matmul), and its partial o_proj; a 4-way ReduceScatter sums the o_proj
partials, each core returning a 512-row slice of o^T for its batch.

Engine balance: PE does only matmuls (f32r for projections/scores/o_proj,
bf16 for P.V), exp runs on the scalar engine (bf16 out), elementwise work is
split between the vector (DVE) and pool (gpsimd) engines, rotate-half swaps
are SBUF DMAs on the pool queue, V transposes use the DMA xbar, and the causal
diagonal is tightened to 512/384/256/256-wide sub-tiles.
"""
import numpy as np
import concourse.bass as bass
import concourse.bacc as bacc
import concourse.tile as tile
import concourse.mybir as mybir
from concourse.bass_utils import run_bass_kernel_spmd

f32 = mybir.dt.float32
f32r = mybir.dt.float32r
bf16 = mybir.dt.bfloat16
AF = mybir.ActivationFunctionType
MUL = mybir.AluOpType.mult
ADD = mybir.AluOpType.add

B, S, HID = 2, 2048, 2048
NH, NKV, HD = 16, 4, 128
GH = NH // NKV          # query heads per core (4)
TB = 512                # token block (q block / projection block)
NT = S // TB            # 4 token blocks
NCT = HID // 128        # 16 contraction tiles
NKT = S // 128          # 16 key tiles

_CACHE = {}
_DEBUG_OUTS = False


def _build(causal: bool, with_rs: bool = True):
    nc = bacc.Bacc("TRN2", target_bir_lowering=False, debug=False, num_devices=8)
    xt = nc.dram_tensor("xt", [HID, S], f32, kind="ExternalInput").ap()
    wq = nc.dram_tensor("wq", [HID, GH * HD], f32, kind="ExternalInput").ap()
    wk = nc.dram_tensor("wk", [HID, HD], f32, kind="ExternalInput").ap()
    wv = nc.dram_tensor("wv", [HID, HD], f32, kind="ExternalInput").ap()
    wo = nc.dram_tensor("wo", [GH * HD, HID], f32, kind="ExternalInput").ap()
    cos_d = nc.dram_tensor("cos_t", [HD, S], f32, kind="ExternalInput").ap()
    sin_d = nc.dram_tensor("sin_m", [HD, S], f32, kind="ExternalInput").ap()
    cm_d = nc.dram_tensor("cmask", [128, 768], f32, kind="ExternalInput").ap()
    on_d = nc.dram_tensor("ones_in", [128, 128], f32, kind="ExternalInput").ap()
    id_d = nc.dram_tensor("ident_in", [128, 128], f32, kind="ExternalInput").ap()
    out_r = nc.dram_tensor("out_r", [TB, S], f32, kind="ExternalOutput").ap()

    with tile.TileContext(nc) as tc:
        with (
            tc.tile_pool(name="glob", bufs=1) as glob,
            tc.tile_pool(name="dram", bufs=1, space="DRAM") as dram,
        ):
            # ---- global resident stores ----
            kt_rope = glob.tile([128, S], f32r, tag="kt")          # roped K^T [d, k]
            v_all = glob.tile([128, S], bf16, tag="v")             # V natural (bf16), 128i:+128 = tile i
            qt_rope = [glob.tile([128, S], f32r, tag=f"q{h}", name=f"qt_rope{h}")
                       for h in range(GH)]
            cm_b = glob.tile([128, 768], bf16, tag="cmb")          # 0/1 keep masks (2 patterns)
            ones_b = glob.tile([128, 128], bf16, tag="onesb")
            ident_b = glob.tile([128, 128], bf16, tag="identb")

            oT_part = dram.tile([HID, S], f32)                     # o^T partial
            oT_red = dram.tile([TB, S], f32)

            # ---- phase A: projections + rope (phase-scoped SBUF) ----
            with tc.tile_pool(name="pA", bufs=1) as pA, \
                 tc.tile_pool(name="psA", bufs=1, space="PSUM") as psA:
                # batched weight loads: DRAM [c, d] -> SBUF [c-sub(128), ci, d].
                # Issue order matters: the DMA engines drain in order, so load
                # what phase A needs first (wk, rope tables), then xt(tb0) is
                # issued inside the loop, then wv/wq, and wo during tb1.
                wk_all = pA.tile([128, NCT, HD], f32r, tag="wk")
                nc.sync.dma_start(wk_all[:, 0:4, :], wk[0:512, :].bitcast(f32r)
                                  .rearrange("(c p) d -> p c d", p=128))
                id_f = pA.tile([128, 128], f32, tag="idf")
                nc.sync.dma_start(id_f[:], id_d[:])
                nc.vector.tensor_copy(ident_b[:], id_f[:])
                warm = pA.tile([128, 1], f32, tag="warm")
                nc.scalar.activation(warm[:], id_f[:, 0:1], AF.Exp)
                wv_all = pA.tile([128, NCT, HD], f32r, tag="wv")
                wq_all = [pA.tile([128, NCT, HD], f32r, tag=f"wqh{h}",
                                  name=f"wq_all{h}") for h in range(GH)]

                def rope(ps, dst_ap, cs, sn, flip):
                    """dst = ps*cos + swap64(ps)*sin_mod for token block tb."""
                    raw = pA.tile([128, TB], f32, tag="raw", bufs=2, name="raw")
                    nc.scalar.copy(raw[:], ps[:])
                    rot = pA.tile([128, TB], f32, tag="rot", bufs=6, name="rot")
                    nc.gpsimd.dma_start(rot[0:64, :], raw[64:128, :])
                    nc.gpsimd.dma_start(rot[64:128, :], raw[0:64, :])
                    m1 = pA.tile([128, TB], f32, tag="m1", bufs=6, name="m1")
                    nc.vector.tensor_tensor(m1[:], ps[:], cs[:], op=MUL)  # PSUM: DVE
                    nc.vector.tensor_tensor(rot[:], rot[:], sn[:], op=MUL)
                    nc.vector.tensor_tensor(dst_ap, m1[:], rot[:], op=ADD)

                for tb in range(NT):
                    cos_s = pA.tile([128, TB], f32, tag="cos", bufs=4, name="cos")
                    sin_s = pA.tile([128, TB], f32, tag="sin", bufs=4, name="sin")
                    if tb != 0:
                        # rope tables just ahead of the xt tiles
                        nc.sync.dma_start(cos_s[:], cos_d[:, TB * tb:TB * (tb + 1)])
                        nc.sync.dma_start(sin_s[:], sin_d[:, TB * tb:TB * (tb + 1)])
                    xt_t = []
                    for ci in range(NCT):
                        t = pA.tile([128, TB], f32r, tag="xt", bufs=32, name="xt")
                        nc.sync.dma_start(
                            t[:], xt[128 * ci:128 * (ci + 1),
                                     TB * tb:TB * (tb + 1)].bitcast(f32r))
                        xt_t.append(t)
                        if tb == 0 and ci % 4 == 3 and ci < 15:
                            c = ci // 4 + 1  # stream wk in behind the xt tiles
                            nc.sync.dma_start(
                                wk_all[:, 4 * c:4 * (c + 1), :],
                                wk[512 * c:512 * (c + 1), :].bitcast(f32r)
                                .rearrange("(c p) d -> p c d", p=128))
                        if tb == 0 and ci == 0:
                            nc.sync.dma_start(cos_s[:], cos_d[:, 0:TB])
                            nc.sync.dma_start(sin_s[:], sin_d[:, 0:TB])
                        if tb == 0 and ci == 10:
                            # wv lands right as the V matmuls want it; K's last
                            # xt tiles shift later but K ends DMA-paced anyway
                            nc.sync.dma_start(wv_all[:], wv[:].bitcast(f32r)
                                              .rearrange("(c p) d -> p c d", p=128))
                    if tb == 0:
                        for h in range(GH):
                            nc.sync.dma_start(
                                wq_all[h][:], wq[:, HD * h:HD * (h + 1)]
                                .bitcast(f32r).rearrange("(c p) d -> p c d", p=128))
                    # K
                    ps_k = psA.tile([128, TB], f32, tag="pk")
                    for ci in range(NCT):
                        nc.tensor.matmul(ps_k[:], wk_all[:, ci, :], xt_t[ci][:],
                                         start=ci == 0, stop=ci == NCT - 1)
                    rope(ps_k, kt_rope[:, TB * tb:TB * (tb + 1)], cos_s, sin_s,
                         flip=True)
                    # V
                    ps_v = psA.tile([128, TB], f32, tag="pv")
                    for ci in range(NCT):
                        nc.tensor.matmul(ps_v[:], wv_all[:, ci, :], xt_t[ci][:],
                                         start=ci == 0, stop=ci == NCT - 1)
                    vt_sb = pA.tile([128, TB], bf16, tag="vts", bufs=2, name="vt_sb")
                    nc.scalar.copy(vt_sb[:], ps_v[:])
                    # Q heads; one V transpose is spread between each pair of
                    # head blocks so the single ptr bank's WAR (on the previous
                    # transpose's drain copy) never stalls the PE
                    for h in range(GH):
                        # ps_q: one shared 5-deep rotation across all heads and
                        # blocks -- the slot reused by Q_h(tb+1) was freed more
                        # than a full block ago, so the 3-engine rope chain of
                        # block tb never back-pressures the next block
                        ps_q = psA.tile([128, TB], f32, tag="pq", bufs=5,
                                        name=f"ps_q{h}")
                        for ci in range(NCT):
                            nc.tensor.matmul(ps_q[:], wq_all[h][:, ci, :], xt_t[ci][:],
                                             start=ci == 0, stop=ci == NCT - 1)
                        u = h
                        ps_tr = psA.tile([128, 128], bf16, tag="ptr", bufs=1,
                                         name="ps_tr")
                        nc.tensor.transpose(ps_tr[:], vt_sb[:, 128 * u:128 * (u + 1)],
                                            ident_b[:])
                        # Act (not DVE): DVE's in-order queue sits behind rope
                        # m1 ops that can wait on the cos/sin loads
                        nc.scalar.copy(
                            v_all[:, 128 * (4 * tb + u):128 * (4 * tb + u + 1)],
                            ps_tr[:])
                        rope(ps_q, qt_rope[h][:, TB * tb:TB * (tb + 1)], cos_s,
                             sin_s, flip=(h % 2 == 0))

            # ---- phase B: attention + partial o_proj (phase-scoped SBUF) ----
            with tc.tile_pool(name="pB", bufs=1) as pB, \
                 tc.tile_pool(name="psB", bufs=1, space="PSUM") as psB:
                # one-time small loads + casts (tiles scoped here, used here)
                cm_f = pB.tile([128, 768], f32, tag="cmf")
                nc.sync.dma_start(cm_f[:], cm_d[:])
                nc.vector.tensor_copy(cm_b[:], cm_f[:])
                on_f = pB.tile([128, 128], f32, tag="onf")
                nc.sync.dma_start(on_f[:], on_d[:])
                nc.vector.tensor_copy(ones_b[:], on_f[:])
                # o_proj weights: first o_proj runs ~25us into phase B, so this
                # load hides behind the first attention block
                wo_all = pB.tile([128, GH * HID], f32r, tag="wo")  # [j-sub, jh*2048+c]
                nc.sync.dma_start(wo_all[:].rearrange("p (h c) -> p h c", h=GH),
                                  wo[:].bitcast(f32r)
                                  .rearrange("(h p) c -> p h c", p=128))

                def oproj_chunk(j, at_j, cb, last=False):
                    """One 4-co-tile chunk of q-block j's o_proj, batched into a
                    single output DMA."""
                    ob4 = pB.tile([128, 4, TB], f32, tag="ob", bufs=3,
                                  name="ob4")
                    for u in range(4):
                        co = 4 * cb + u
                        ps_p = psB.tile([128, TB], f32, tag="ps_d", bufs=2,
                                        name="ps_p")
                        for jh in range(GH):
                            nc.tensor.matmul(ps_p[:],
                                             wo_all[:, jh * HID + 128 * co:
                                                    jh * HID + 128 * (co + 1)],
                                             at_j[jh][:], start=(jh == 0),
                                             stop=(jh == GH - 1))
                        if u % 2 == 1:
                            nc.scalar.copy(ob4[:, u, :], ps_p[:])
                        else:
                            nc.vector.tensor_copy(ob4[:, u, :], ps_p[:])
                    if cb == 0 and not with_rs:
                        # no-collective mode returns oT_part[0:512] as out_r;
                        # write it there directly
                        dst = out_r[:, TB * j:TB * (j + 1)]
                    else:
                        dst = oT_part[512 * cb:512 * (cb + 1),
                                      TB * j:TB * (j + 1)]
                    if last:
                        # two half DMAs so the final transfer tail is shorter
                        d3 = dst.rearrange("(u p) t -> p u t", p=128)
                        nc.sync.dma_start(d3[:, 0:2, :], ob4[:, 0:2, :])
                        nc.sync.dma_start(d3[:, 2:4, :], ob4[:, 2:4, :])
                    else:
                        nc.sync.dma_start(
                            dst.rearrange("(u p) t -> p u t", p=128), ob4[:])

                pending = []  # deferred o_proj chunk closures
                # q-blocks in rotated order so the final (tail) o_proj pairs
                # with the SHORT j=0 block: its at_s chain hides behind the
                # previous block's o_proj instead of stalling the PE
                for j in ([1, 2, 3, 0] if causal else range(NT)):
                    # (i, q-col-offset) tile list: full tiles then tightened diagonal
                    if causal:
                        tiles = [(i, 0) for i in range(4 * j)]
                        tiles += [(4 * j + m, min(128 * m, 256)) for m in range(4)]
                    else:
                        tiles = [(i, 0) for i in range(NKT)]
                    last_i = tiles[-1][0]

                    ps_o = [psB.tile([128, TB], f32, tag="po", bufs=4, name=f"ps_o{h}")
                            for h in range(GH)]
                    acc = [pB.tile([128, TB], bf16, tag=f"acc{h}", bufs=1,
                                   name=f"acch{h}") for h in range(GH)]
                    av_pend = []  # (h, i, off, w, pt) AVs deferred from i<2

                    def flush_av():
                        for (fh, fi, foff, fw, fpt) in av_pend:
                            nc.tensor.matmul(ps_o[fh][:, foff:TB],
                                             v_all[:, 128 * fi:128 * (fi + 1)],
                                             fpt[:, 0:fw],
                                             start=(fi == 0), stop=(fi == last_i),
                                             skip_group_check=True)
                        av_pend.clear()

                    for ti, (i, off) in enumerate(tiles):
                        w = TB - off
                        diag = causal and i >= 4 * j
                        m = i - 4 * j if diag else -1
                        if i == 3:
                            # the i<3 AVs were deferred so the first PSUM write
                            # to the po banks (WAR on last block's at_s) comes
                            # after ~12 scores' worth of PE work
                            flush_av()
                        for h in range(GH):
                            ps_s = psB.tile([128, TB], f32, tag="ps_s", bufs=2,
                                            name="ps_s")
                            nc.tensor.matmul(ps_s[:, 0:w],
                                             kt_rope[:, 128 * i:128 * (i + 1)],
                                             qt_rope[h][:, TB * j + off:TB * (j + 1)],
                                             start=True, stop=True)
                            pt = pB.tile([128, TB], bf16, tag="pt", bufs=16, name="pt")
                            nc.scalar.activation(pt[:, 0:w], ps_s[:, 0:w], AF.Exp)
                            if diag:
                                patt = 512 if m == 3 else 0
                                meng = nc.vector
                                meng.tensor_tensor(
                                    pt[:, 0:w], pt[:, 0:w],
                                    cm_b[:, patt:patt + w], op=MUL)
                            if i == 0:
                                nc.vector.tensor_copy(acc[h][:], pt[:])
                            else:
                                nc.vector.tensor_tensor(acc[h][:, off:TB],
                                                        acc[h][:, off:TB],
                                                        pt[:, 0:w], op=ADD)
                            if i < 3:
                                av_pend.append((h, i, off, w, pt))
                                continue
                            nc.tensor.matmul(ps_o[h][:, off:TB],
                                             v_all[:, 128 * i:128 * (i + 1)],
                                             pt[:, 0:w],
                                             start=False, stop=(i == last_i),
                                             skip_group_check=True)
                    flush_av()
                    # any previous-block o_proj chunks not yet interleaved
                    while pending:
                        pending.pop(0)()

                    # normalize into A^T blocks
                    at_s = [pB.tile([128, TB], f32r, tag=f"at{h}", bufs=1,
                                    name=f"at_s{h}") for h in range(GH)]
                    for h in range(GH):
                        ps_d = psB.tile([128, TB], f32, tag="ps_d", bufs=2,
                                        name="ps_d")
                        nc.tensor.matmul(ps_d[:], ones_b[:], acc[h][:],
                                         start=True, stop=True)
                        rec = pB.tile([128, TB], f32, tag="rec", bufs=2, name="rec")
                        nc.vector.reciprocal(rec[:], ps_d[:])
                        nc.vector.tensor_tensor(at_s[h][:], ps_o[h][:], rec[:],
                                                op=MUL)  # PSUM: DVE only
                    pending = [
                        (lambda jj, aa, cc: lambda la=False: oproj_chunk(
                            jj, aa, cc, la))(j, at_s, cb)
                        for cb in range(NCT // 4)]
                while len(pending) > 1:
                    pending.pop(0)()
                pending.pop(0)(True)

            if _DEBUG_OUTS:
                dbg_kt = nc.dram_tensor("dbg_kt", [128, S], f32,
                                        kind="ExternalOutput").ap()
                dbg_q0 = nc.dram_tensor("dbg_q0", [128, S], f32,
                                        kind="ExternalOutput").ap()
                dbg_v = nc.dram_tensor("dbg_v", [128, S // 2], f32,
                                       kind="ExternalOutput").ap()
                nc.sync.dma_start(dbg_kt[:], kt_rope[:].bitcast(f32))
                nc.sync.dma_start(dbg_q0[:], qt_rope[0][:].bitcast(f32))
                nc.sync.dma_start(dbg_v[:], v_all[:].bitcast(f32))  # raw bits

            # ---- phase C: ReduceScatter partials, emit this core's slice ----
            if with_rs:
                nc.gpsimd.collective_compute(
                    "ReduceScatter", ADD,
                    replica_groups=[[0, 1, 2, 3], [4, 5, 6, 7]],
                    ins=[oT_part[:].opt()], outs=[oT_red[:].opt()],
                )
                nc.sync.dma_start(out_r[:], oT_red[:])
            # else: emit_oproj already wrote out_r directly

    nc.compile()
    return nc


def kernel(hidden_states, attention_mask, Wq, Wk, Wv, Wo, sin, cos):
    hidden_states = np.asarray(hidden_states, dtype=np.float32)
    attention_mask = np.asarray(attention_mask, dtype=np.float32)
    Wq, Wk, Wv, Wo = (np.ascontiguousarray(np.asarray(a, dtype=np.float32))
                      for a in (Wq, Wk, Wv, Wo))
    sin = np.asarray(sin, dtype=np.float32)
    cos = np.asarray(cos, dtype=np.float32)

    # classify the mask: causal (top-right strictly very-negative, elsewhere 0,
    # col 0 ignored since reference zeroes it) vs all-zeros (full attention)
    m0 = attention_mask[0, 0]
    iu = np.triu_indices(S, k=1)
    causal = bool((m0[iu] < -1e30).all() and
                  (m0[np.tril_indices(S, k=0)] == 0.0).all())
    if not causal:
        assert (attention_mask == 0).all(), "unsupported attention mask pattern"
    if causal:
        for b in range(1, B):
            assert np.array_equal(attention_mask[b, 0], m0), "mask differs per batch"

    key = causal
    if key not in _CACHE:
        _CACHE[key] = _build(causal)
    nc = _CACHE[key]

    cos_t = np.ascontiguousarray(cos[:S].T)          # [128, S]
    sin_t = cos_t.copy()
    sin_t[:] = sin[:S].T
    sin_m = sin_t.copy()
    sin_m[:64] *= -1.0
    # 0/1 causal keep-patterns: patt0 = (q >= k), patt1 = (q >= k + 128)
    kl = np.arange(128)[:, None]
    ql = np.arange(512)[None, :]
    cmask = np.concatenate(
        [(ql >= kl).astype(np.float32),
         (ql[:, :256] >= kl + 128).astype(np.float32)], axis=1)

    in_maps = []
    for c in range(8):
        b, g = c // 4, c % 4
        in_maps.append({
            "xt": np.ascontiguousarray(hidden_states[b].T),
            "wq": np.ascontiguousarray(Wq[512 * g:512 * (g + 1), :].T),
            "wk": np.ascontiguousarray(Wk[128 * g:128 * (g + 1), :].T),
            "wv": np.ascontiguousarray(Wv[128 * g:128 * (g + 1), :].T),
            "wo": np.ascontiguousarray(Wo[:, 512 * g:512 * (g + 1)].T),
            "cos_t": cos_t, "sin_m": sin_m, "cmask": cmask,
            "ones_in": np.ones((128, 128), dtype=np.float32),
            "ident_in": np.eye(128, dtype=np.float32),
        })

    global _LAST_IN_MAPS, _LAST_RES
    _LAST_IN_MAPS = in_maps
    res = run_bass_kernel_spmd(nc, in_maps, core_ids=list(range(8)))
    _LAST_RES = res

    out = np.empty((B, S, HID), dtype=np.float32)
    for c in range(8):
        b, r = c // 4, c % 4
        out[b, :, TB * r:TB * (r + 1)] = res.results[c]["out_r"].T
    return out


if __name__ == "__main__":
    rng = np.random.default_rng(0)
    h = rng.standard_normal((B, S, HID), dtype=np.float32)
    print("module loads ok")


# revision 59
# speedup vs baseline: 1.4360x; 1.0181x over previous
"""HF OpenMoe attention (B=2,S=2048,HID=2048,NH=16,NKV=4,HD=128) on 8 trn2 cores.

Sharding: core c -> (batch b=c//4, kv-group g=c%4). Each core computes Q/K/V
projections for its 4 query heads + 1 kv head, RoPE, causal flash attention in
S^T layout (scores transposed: [k, q], softmax over the partition dim via
ones-matmul), and its partial o_proj; a 4-way ReduceScatter sums the o_proj
partials, each core returning a 512-row slice of o^T for its batch.

Engine balance: PE does only matmuls (f32r for projections/scores/o_proj,
bf16 for P.V and V-transposes), exp runs on the scalar engine (bf16 out),
rope/softmax elementwise work is spread over DVE/Pool/Act, rotate-half swaps
are SBUF DMAs on the pool queue, and the causal diagonal is tightened to
512/384/256/256-wide sub-tiles. Weight/x DMAs are batched via 3D access
patterns and ordered so compute starts as soon as the first tiles land;
o_proj of block j is emitted during block j+1's attention; the i<3 P.V
accumulations are deferred past the scores so PSUM-bank reuse never stalls
the PE at block handoffs.
"""
import numpy as np
import concourse.bass as bass
import concourse.bacc as bacc
import concourse.tile as tile
import concourse.mybir as mybir
from concourse.bass_utils import run_bass_kernel_spmd

f32 = mybir.dt.float32
f32r = mybir.dt.float32r
bf16 = mybir.dt.bfloat16
AF = mybir.ActivationFunctionType
MUL = mybir.AluOpType.mult
ADD = mybir.AluOpType.add

B, S, HID = 2, 2048, 2048
NH, NKV, HD = 16, 4, 128
GH = NH // NKV          # query heads per core (4)
TB = 512                # token block (q block / projection block)
NT = S // TB            # 4 token blocks
NCT = HID // 128        # 16 contraction tiles
NKT = S // 128          # 16 key tiles

_CACHE = {}
_DEBUG_OUTS = False


def _build(causal: bool, with_rs: bool = True):
    nc = bacc.Bacc("TRN2", target_bir_lowering=False, debug=False, num_devices=8)
    xt = nc.dram_tensor("xt", [HID, S], f32, kind="ExternalInput").ap()
    wq = nc.dram_tensor("wq", [HID, GH * HD], f32, kind="ExternalInput").ap()
    wk = nc.dram_tensor("wk", [HID, HD], f32, kind="ExternalInput").ap()
    wv = nc.dram_tensor("wv", [HID, HD], f32, kind="ExternalInput").ap()
    wo = nc.dram_tensor("wo", [GH * HD, HID], f32, kind="ExternalInput").ap()
    cos_d = nc.dram_tensor("cos_t", [HD, S], f32, kind="ExternalInput").ap()
    sin_d = nc.dram_tensor("sin_m", [HD, S], f32, kind="ExternalInput").ap()
    cm_d = nc.dram_tensor("cmask", [128, 768], f32, kind="ExternalInput").ap()
    on_d = nc.dram_tensor("ones_in", [128, 128], f32, kind="ExternalInput").ap()
    id_d = nc.dram_tensor("ident_in", [128, 128], f32, kind="ExternalInput").ap()
    out_r = nc.dram_tensor("out_r", [TB, S], f32, kind="ExternalOutput").ap()

    with tile.TileContext(nc) as tc:
        with (
            tc.tile_pool(name="glob", bufs=1) as glob,
            tc.tile_pool(name="dram", bufs=1, space="DRAM") as dram,
        ):
            # ---- global resident stores ----
            kt_rope = glob.tile([128, S], f32r, tag="kt")          # roped K^T [d, k]
            v_all = glob.tile([128, S], bf16, tag="v")             # V natural (bf16), 128i:+128 = tile i
            qt_rope = [glob.tile([128, S], f32r, tag=f"q{h}", name=f"qt_rope{h}")
                       for h in range(GH)]
            cm_b = glob.tile([128, 768], bf16, tag="cmb")          # 0/1 keep masks (2 patterns)
            ones_b = glob.tile([128, 128], bf16, tag="onesb")
            ident_b = glob.tile([128, 128], bf16, tag="identb")

            oT_part = dram.tile([HID, S], f32)                     # o^T partial
            oT_red = dram.tile([TB, S], f32)

            # ---- phase A: projections + rope (phase-scoped SBUF) ----
            with tc.tile_pool(name="pA", bufs=1) as pA, \
                 tc.tile_pool(name="psA", bufs=1, space="PSUM") as psA:
                # batched weight loads: DRAM [c, d] -> SBUF [c-sub(128), ci, d].
                # Issue order matters: the DMA engines drain in order, so wk
                # comes first (chunked between xt tiles), wv/wq mid-stream,
                # and wo not until phase B.
                wk_all = pA.tile([128, NCT, HD], f32r, tag="wk")
                nc.sync.dma_start(wk_all[:, 0:4, :], wk[0:512, :].bitcast(f32r)
                                  .rearrange("(c p) d -> p c d", p=128))
                wv_all = pA.tile([128, NCT, HD], f32r, tag="wv")
                wq_all = [pA.tile([128, NCT, HD], f32r, tag=f"wqh{h}",
                                  name=f"wq_all{h}") for h in range(GH)]

                def rope(ps, dst_ap, cs, sn, flip):
                    """dst = ps*cos + swap64(ps)*sin_mod for token block tb."""
                    raw = pA.tile([128, TB], f32, tag="raw", bufs=2, name="raw")
                    nc.scalar.copy(raw[:], ps[:])
                    rot = pA.tile([128, TB], f32, tag="rot", bufs=6, name="rot")
                    nc.gpsimd.dma_start(rot[0:64, :], raw[64:128, :])
                    nc.gpsimd.dma_start(rot[64:128, :], raw[0:64, :])
                    m1 = pA.tile([128, TB], f32, tag="m1", bufs=6, name="m1")
                    nc.vector.tensor_tensor(m1[:], ps[:], cs[:], op=MUL)  # PSUM: DVE
                    nc.vector.tensor_tensor(rot[:], rot[:], sn[:], op=MUL)
                    nc.vector.tensor_tensor(dst_ap, m1[:], rot[:], op=ADD)

                for tb in range(NT):
                    cos_s = pA.tile([128, TB], f32, tag="cos", bufs=4, name="cos")
                    sin_s = pA.tile([128, TB], f32, tag="sin", bufs=4, name="sin")
                    if tb != 0:
                        # rope tables just ahead of the xt tiles
                        nc.sync.dma_start(cos_s[:], cos_d[:, TB * tb:TB * (tb + 1)])
                        nc.sync.dma_start(sin_s[:], sin_d[:, TB * tb:TB * (tb + 1)])
                    xt_t = []
                    for ci in range(NCT):
                        t = pA.tile([128, TB], f32r, tag="xt", bufs=32, name="xt")
                        nc.sync.dma_start(
                            t[:], xt[128 * ci:128 * (ci + 1),
                                     TB * tb:TB * (tb + 1)].bitcast(f32r))
                        xt_t.append(t)
                        if tb == 0 and ci % 4 == 3 and ci < 15:
                            c = ci // 4 + 1  # stream wk in behind the xt tiles
                            nc.sync.dma_start(
                                wk_all[:, 4 * c:4 * (c + 1), :],
                                wk[512 * c:512 * (c + 1), :].bitcast(f32r)
                                .rearrange("(c p) d -> p c d", p=128))
                        if tb == 0 and ci == 0:
                            nc.sync.dma_start(cos_s[:], cos_d[:, 0:TB])
                            nc.sync.dma_start(sin_s[:], sin_d[:, 0:TB])
                        if tb == 0 and ci == 1:
                            id_f = pA.tile([128, 128], f32, tag="idf")
                            nc.sync.dma_start(id_f[:], id_d[:])
                            nc.vector.tensor_copy(ident_b[:], id_f[:])
                            warm = pA.tile([128, 1], f32, tag="warm")
                            nc.scalar.activation(warm[:], id_f[:, 0:1], AF.Exp)
                        if tb == 0 and ci == 10:
                            # wv lands right as the V matmuls want it; K's last
                            # xt tiles shift later but K ends DMA-paced anyway
                            nc.sync.dma_start(wv_all[:], wv[:].bitcast(f32r)
                                              .rearrange("(c p) d -> p c d", p=128))
                    if tb == 0:
                        for h in range(GH):
                            nc.sync.dma_start(
                                wq_all[h][:], wq[:, HD * h:HD * (h + 1)]
                                .bitcast(f32r).rearrange("(c p) d -> p c d", p=128))
                    # K
                    ps_k = psA.tile([128, TB], f32, tag="pk")
                    for ci in range(NCT):
                        nc.tensor.matmul(ps_k[:], wk_all[:, ci, :], xt_t[ci][:],
                                         start=ci == 0, stop=ci == NCT - 1)
                    rope(ps_k, kt_rope[:, TB * tb:TB * (tb + 1)], cos_s, sin_s,
                         flip=True)

                    def emit_v():
                        ps_v = psA.tile([128, TB], f32, tag="pv")
                        for ci in range(NCT):
                            nc.tensor.matmul(ps_v[:], wv_all[:, ci, :],
                                             xt_t[ci][:],
                                             start=ci == 0, stop=ci == NCT - 1)
                        vt_sb = pA.tile([128, TB], bf16, tag="vts", bufs=2,
                                        name="vt_sb")
                        nc.scalar.copy(vt_sb[:], ps_v[:])
                        return vt_sb

                    def emit_vtrans(vt_sb, u):
                        ps_tr = psA.tile([128, 128], bf16, tag="ptr", bufs=1,
                                         name="ps_tr")
                        nc.tensor.transpose(ps_tr[:],
                                            vt_sb[:, 128 * u:128 * (u + 1)],
                                            ident_b[:])
                        # Act (not DVE): DVE's in-order queue sits behind rope
                        # m1 ops that can wait on the cos/sin loads
                        nc.scalar.copy(
                            v_all[:, 128 * (4 * tb + u):128 * (4 * tb + u + 1)],
                            ps_tr[:])

                    vt_sb = emit_v() if tb < NT - 1 else None
                    # Q heads; one V transpose is spread between each pair of
                    # head blocks so the single ptr bank's WAR (on the previous
                    # transpose's drain copy) never stalls the PE
                    for h in range(GH):
                        ps_q = psA.tile([128, TB], f32, tag="pq", bufs=5,
                                        name=f"ps_q{h}")
                        for ci in range(NCT):
                            nc.tensor.matmul(ps_q[:], wq_all[h][:, ci, :],
                                             xt_t[ci][:],
                                             start=ci == 0, stop=ci == NCT - 1)
                        if vt_sb is not None:
                            emit_vtrans(vt_sb, h)
                        rope(ps_q, qt_rope[h][:, TB * tb:TB * (tb + 1)], cos_s,
                             sin_s, flip=(h % 2 == 0))
                    if vt_sb is None:
                        # last block: V after the Q heads, hiding the final
                        # rope chain's latency behind V's matmuls
                        vt_sb = emit_v()
                        for u in range(4):
                            emit_vtrans(vt_sb, u)

            # ---- phase B: attention + partial o_proj (phase-scoped SBUF) ----
            with tc.tile_pool(name="pB", bufs=1) as pB, \
                 tc.tile_pool(name="psB", bufs=1, space="PSUM") as psB:
                # one-time small loads + casts (tiles scoped here, used here)
                cm_f = pB.tile([128, 768], f32, tag="cmf")
                nc.sync.dma_start(cm_f[:], cm_d[:])
                nc.vector.tensor_copy(cm_b[:], cm_f[:])
                on_f = pB.tile([128, 128], f32, tag="onf")
                nc.sync.dma_start(on_f[:], on_d[:])
                nc.vector.tensor_copy(ones_b[:], on_f[:])
                # o_proj weights: first o_proj runs ~25us into phase B, so this
                # load hides behind the first attention block
                wo_all = pB.tile([128, GH * HID], f32r, tag="wo")  # [j-sub, jh*2048+c]
                nc.sync.dma_start(wo_all[:].rearrange("p (h c) -> p h c", h=GH),
                                  wo[:].bitcast(f32r)
                                  .rearrange("(h p) c -> p h c", p=128))

                def oproj_chunk(j, at_j, cb, last=False):
                    """One 4-co-tile chunk of q-block j's o_proj, batched into a
                    single output DMA."""
                    ob4 = pB.tile([128, 4, TB], f32, tag="ob", bufs=3,
                                  name="ob4")
                    for u in range(4):
                        co = 4 * cb + u
                        ps_p = psB.tile([128, TB], f32, tag="ps_d", bufs=2,
                                        name="ps_p")
                        for jh in range(GH):
                            nc.tensor.matmul(ps_p[:],
                                             wo_all[:, jh * HID + 128 * co:
                                                    jh * HID + 128 * (co + 1)],
                                             at_j[jh][:], start=(jh == 0),
                                             stop=(jh == GH - 1))
                        if u % 2 == 1:
                            nc.scalar.copy(ob4[:, u, :], ps_p[:])
                        else:
                            nc.vector.tensor_copy(ob4[:, u, :], ps_p[:])
                    if cb == 0 and not with_rs:
                        # no-collective mode returns oT_part[0:512] as out_r;
                        # write it there directly
                        dst = out_r[:, TB * j:TB * (j + 1)]
                    else:
                        dst = oT_part[512 * cb:512 * (cb + 1),
                                      TB * j:TB * (j + 1)]
                    if last:
                        # two half DMAs so the final transfer tail is shorter
                        d3 = dst.rearrange("(u p) t -> p u t", p=128)
                        nc.sync.dma_start(d3[:, 0:2, :], ob4[:, 0:2, :])
                        nc.sync.dma_start(d3[:, 2:4, :], ob4[:, 2:4, :])
                    else:
                        nc.sync.dma_start(
                            dst.rearrange("(u p) t -> p u t", p=128), ob4[:])

                pending = []  # deferred o_proj chunk closures
                # q-blocks in rotated order so the final (tail) o_proj pairs
                # with the SHORT j=0 block: its at_s chain hides behind the
                # previous block's o_proj instead of stalling the PE
                for j in ([1, 2, 3, 0] if causal else range(NT)):
                    # (i, q-col-offset) tile list: full tiles then tightened diagonal
                    if causal:
                        tiles = [(i, 0) for i in range(4 * j)]
                        tiles += [(4 * j + m, min(128 * m, 256)) for m in range(4)]
                    else:
                        tiles = [(i, 0) for i in range(NKT)]
                    last_i = tiles[-1][0]

                    ps_o = [psB.tile([128, TB], f32, tag="po", bufs=4, name=f"ps_o{h}")
                            for h in range(GH)]
                    acc = [pB.tile([128, TB], bf16, tag=f"acc{h}", bufs=1,
                                   name=f"acch{h}") for h in range(GH)]
                    av_pend = []  # (h, i, off, w, pt) AVs deferred from i<2

                    def flush_av():
                        for (fh, fi, foff, fw, fpt) in av_pend:
                            nc.tensor.matmul(ps_o[fh][:, foff:TB],
                                             v_all[:, 128 * fi:128 * (fi + 1)],
                                             fpt[:, 0:fw],
                                             start=(fi == 0), stop=(fi == last_i),
                                             skip_group_check=True)
                        av_pend.clear()

                    for ti, (i, off) in enumerate(tiles):
                        w = TB - off
                        diag = causal and i >= 4 * j
                        m = i - 4 * j if diag else -1
                        if i == 3:
                            # the i<3 AVs were deferred so the first PSUM write
                            # to the po banks (WAR on last block's at_s) comes
                            # after ~12 scores' worth of PE work
                            flush_av()
                        for h in range(GH):
                            ps_s = psB.tile([128, TB], f32, tag="ps_s", bufs=2,
                                            name="ps_s")
                            nc.tensor.matmul(ps_s[:, 0:w],
                                             kt_rope[:, 128 * i:128 * (i + 1)],
                                             qt_rope[h][:, TB * j + off:TB * (j + 1)],
                                             start=True, stop=True)
                            pt = pB.tile([128, TB], bf16, tag="pt", bufs=16, name="pt")
                            nc.scalar.activation(pt[:, 0:w], ps_s[:, 0:w], AF.Exp)
                            if diag:
                                patt = 512 if m == 3 else 0
                                meng = nc.vector
                                meng.tensor_tensor(
                                    pt[:, 0:w], pt[:, 0:w],
                                    cm_b[:, patt:patt + w], op=MUL)
                            if i == 0:
                                nc.vector.tensor_copy(acc[h][:], pt[:])
                            else:
                                nc.vector.tensor_tensor(acc[h][:, off:TB],
                                                        acc[h][:, off:TB],
                                                        pt[:, 0:w], op=ADD)
                            if i < 3:
                                av_pend.append((h, i, off, w, pt))
                                continue
                            nc.tensor.matmul(ps_o[h][:, off:TB],
                                             v_all[:, 128 * i:128 * (i + 1)],
                                             pt[:, 0:w],
                                             start=False, stop=(i == last_i),
                                             skip_group_check=True)
                    flush_av()
                    # any previous-block o_proj chunks not yet interleaved
                    while pending:
                        pending.pop(0)()

                    # normalize into A^T blocks
                    at_s = [pB.tile([128, TB], f32r, tag=f"at{h}", bufs=1,
                                    name=f"at_s{h}") for h in range(GH)]
                    for h in range(GH):
                        ps_d = psB.tile([128, TB], f32, tag="ps_d", bufs=2,
                                        name="ps_d")
                        nc.tensor.matmul(ps_d[:], ones_b[:], acc[h][:],
                                         start=True, stop=True)
                        rec = pB.tile([128, TB], f32, tag="rec", bufs=2, name="rec")
                        nc.vector.reciprocal(rec[:], ps_d[:])
                        nc.vector.tensor_tensor(at_s[h][:], ps_o[h][:], rec[:],
                                                op=MUL)  # PSUM: DVE only
                    pending = [
                        (lambda jj, aa, cc: lambda la=False: oproj_chunk(
                            jj, aa, cc, la))(j, at_s, cb)
                        for cb in range(NCT // 4)]
                while len(pending) > 2:
                    pending.pop(0)()
                pending.pop(0)(True)
                pending.pop(0)(True)

            if _DEBUG_OUTS:
                dbg_kt = nc.dram_tensor("dbg_kt", [128, S], f32,
                                        kind="ExternalOutput").ap()
                dbg_q0 = nc.dram_tensor("dbg_q0", [128, S], f32,
                                        kind="ExternalOutput").ap()
                dbg_v = nc.dram_tensor("dbg_v", [128, S // 2], f32,
                                       kind="ExternalOutput").ap()
                nc.sync.dma_start(dbg_kt[:], kt_rope[:].bitcast(f32))
                nc.sync.dma_start(dbg_q0[:], qt_rope[0][:].bitcast(f32))
                nc.sync.dma_start(dbg_v[:], v_all[:].bitcast(f32))  # raw bits

            # ---- phase C: ReduceScatter partials, emit this core's slice ----
            if with_rs:
                nc.gpsimd.collective_compute(
                    "ReduceScatter", ADD,
                    replica_groups=[[0, 1, 2, 3], [4, 5, 6, 7]],
                    ins=[oT_part[:].opt()], outs=[oT_red[:].opt()],
                )
                nc.sync.dma_start(out_r[:], oT_red[:])
            # else: emit_oproj already wrote out_r directly

    nc.compile()
    return nc


def kernel(hidden_states, attention_mask, Wq, Wk, Wv, Wo, sin, cos):
    hidden_states = np.asarray(hidden_states, dtype=np.float32)
    attention_mask = np.asarray(attention_mask, dtype=np.float32)
    Wq, Wk, Wv, Wo = (np.ascontiguousarray(np.asarray(a, dtype=np.float32))
                      for a in (Wq, Wk, Wv, Wo))
    sin = np.asarray(sin, dtype=np.float32)
    cos = np.asarray(cos, dtype=np.float32)

    # classify the mask: causal (top-right strictly very-negative, elsewhere 0,
    # col 0 ignored since reference zeroes it) vs all-zeros (full attention)
    m0 = attention_mask[0, 0]
    iu = np.triu_indices(S, k=1)
    causal = bool((m0[iu] < -1e30).all() and
                  (m0[np.tril_indices(S, k=0)] == 0.0).all())
    if not causal:
        assert (attention_mask == 0).all(), "unsupported attention mask pattern"
    if causal:
        for b in range(1, B):
            assert np.array_equal(attention_mask[b, 0], m0), "mask differs per batch"

    key = causal
    if key not in _CACHE:
        _CACHE[key] = _build(causal)
    nc = _CACHE[key]

    cos_t = np.ascontiguousarray(cos[:S].T)          # [128, S]
    sin_t = cos_t.copy()
    sin_t[:] = sin[:S].T
    sin_m = sin_t.copy()
    sin_m[:64] *= -1.0
    # 0/1 causal keep-patterns: patt0 = (q >= k), patt1 = (q >= k + 128)
    kl = np.arange(128)[:, None]
    ql = np.arange(512)[None, :]
    cmask = np.concatenate(
        [(ql >= kl).astype(np.float32),
         (ql[:, :256] >= kl + 128).astype(np.float32)], axis=1)

    in_maps = []
    for c in range(8):
        b, g = c // 4, c % 4
        in_maps.append({
            "xt": np.ascontiguousarray(hidden_states[b].T),
            "wq": np.ascontiguousarray(Wq[512 * g:512 * (g + 1), :].T),
            "wk": np.ascontiguousarray(Wk[128 * g:128 * (g + 1), :].T),
            "wv": np.ascontiguousarray(Wv[128 * g:128 * (g + 1), :].T),
            "wo": np.ascontiguousarray(Wo[:, 512 * g:512 * (g + 1)].T),
            "cos_t": cos_t, "sin_m": sin_m, "cmask": cmask,
            "ones_in": np.ones((128, 128), dtype=np.float32),
            "ident_in": np.eye(128, dtype=np.float32),
        })

    global _LAST_IN_MAPS, _LAST_RES
    _LAST_IN_MAPS = in_maps
    res = run_bass_kernel_spmd(nc, in_maps, core_ids=list(range(8)))
    _LAST_RES = res

    out = np.empty((B, S, HID), dtype=np.float32)
    for c in range(8):
        b, r = c // 4, c % 4
        out[b, :, TB * r:TB * (r + 1)] = res.results[c]["out_r"].T
    return out


if __name__ == "__main__":
    rng = np.random.default_rng(0)
    h = rng.standard_normal((B, S, HID), dtype=np.float32)
    print("module loads ok")


# revision 61
# speedup vs baseline: 1.4578x; 1.0151x over previous
"""HF OpenMoe attention (B=2,S=2048,HID=2048,NH=16,NKV=4,HD=128) on 8 trn2 cores.

Sharding: core c -> (batch b=c//4, kv-group g=c%4). Each core computes Q/K/V
projections for its 4 query heads + 1 kv head, RoPE, causal flash attention in
S^T layout (scores transposed: [k, q], softmax over the partition dim via
ones-matmul), and its partial o_proj; a 4-way ReduceScatter sums the o_proj
partials, each core returning a 512-row slice of o^T for its batch.

Engine balance: PE does only matmuls (f32r for projections/scores/o_proj,
bf16 for P.V and V-transposes), exp runs on the scalar engine (bf16 out),
rope/softmax elementwise work is spread over DVE/Pool/Act, rotate-half swaps
are SBUF DMAs on the pool queue, and the causal diagonal is tightened to
512/384/256/256-wide sub-tiles. Weight/x DMAs are batched via 3D access
patterns and ordered so compute starts as soon as the first tiles land;
o_proj of block j is emitted during block j+1's attention; the i<3 P.V
accumulations are deferred past the scores so PSUM-bank reuse never stalls
the PE at block handoffs.
"""
import numpy as np
import concourse.bass as bass
import concourse.bacc as bacc
import concourse.tile as tile
import concourse.mybir as mybir
from concourse.bass_utils import run_bass_kernel_spmd

f32 = mybir.dt.float32
f32r = mybir.dt.float32r
bf16 = mybir.dt.bfloat16
AF = mybir.ActivationFunctionType
MUL = mybir.AluOpType.mult
ADD = mybir.AluOpType.add

B, S, HID = 2, 2048, 2048
NH, NKV, HD = 16, 4, 128
GH = NH // NKV          # query heads per core (4)
TB = 512                # token block (q block / projection block)
NT = S // TB            # 4 token blocks
NCT = HID // 128        # 16 contraction tiles
NKT = S // 128          # 16 key tiles

_CACHE = {}
_DEBUG_OUTS = False


def _build(causal: bool, with_rs: bool = True):
    nc = bacc.Bacc("TRN2", target_bir_lowering=False, debug=False, num_devices=8)
    xt = nc.dram_tensor("xt", [HID, S], f32, kind="ExternalInput").ap()
    wq = nc.dram_tensor("wq", [HID, GH * HD], f32, kind="ExternalInput").ap()
    wk = nc.dram_tensor("wk", [HID, HD], f32, kind="ExternalInput").ap()
    wv = nc.dram_tensor("wv", [HID, HD], f32, kind="ExternalInput").ap()
    wo = nc.dram_tensor("wo", [GH * HD, HID], f32, kind="ExternalInput").ap()
    cos_d = nc.dram_tensor("cos_t", [HD, S], f32, kind="ExternalInput").ap()
    sin_d = nc.dram_tensor("sin_m", [HD, S], f32, kind="ExternalInput").ap()
    cm_d = nc.dram_tensor("cmask", [128, 768], f32, kind="ExternalInput").ap()
    on_d = nc.dram_tensor("ones_in", [128, 128], f32, kind="ExternalInput").ap()
    id_d = nc.dram_tensor("ident_in", [128, 128], f32, kind="ExternalInput").ap()
    out_r = nc.dram_tensor("out_r", [TB, S], f32, kind="ExternalOutput").ap()

    with tile.TileContext(nc) as tc:
        with (
            tc.tile_pool(name="glob", bufs=1) as glob,
            tc.tile_pool(name="dram", bufs=1, space="DRAM") as dram,
        ):
            # ---- global resident stores ----
            kt_rope = glob.tile([128, S], f32r, tag="kt")          # roped K^T [d, k]
            v_all = glob.tile([128, S], bf16, tag="v")             # V natural (bf16), 128i:+128 = tile i
            qt_rope = [glob.tile([128, S], f32r, tag=f"q{h}", name=f"qt_rope{h}")
                       for h in range(GH)]
            cm_b = glob.tile([128, 768], bf16, tag="cmb")          # 0/1 keep masks (2 patterns)
            ones_b = glob.tile([128, 128], bf16, tag="onesb")
            ident_b = glob.tile([128, 128], bf16, tag="identb")

            oT_part = dram.tile([HID, S], f32)                     # o^T partial
            oT_red = dram.tile([TB, S], f32)

            # ---- phase A: projections + rope (phase-scoped SBUF) ----
            with tc.tile_pool(name="pA", bufs=1) as pA, \
                 tc.tile_pool(name="psA", bufs=1, space="PSUM") as psA:
                # batched weight loads: DRAM [c, d] -> SBUF [c-sub(128), ci, d].
                # Issue order matters: the DMA engines drain in order, so wk
                # comes first (chunked between xt tiles), wv/wq mid-stream,
                # and wo not until phase B.
                wk_all = pA.tile([128, NCT, HD], f32r, tag="wk")
                nc.sync.dma_start(wk_all[:, 0:4, :], wk[0:512, :].bitcast(f32r)
                                  .rearrange("(c p) d -> p c d", p=128))
                wv_all = pA.tile([128, NCT, HD], f32r, tag="wv")
                wq_all = [pA.tile([128, NCT, HD], f32r, tag=f"wqh{h}",
                                  name=f"wq_all{h}") for h in range(GH)]

                def rope(ps, dst_ap, cs, sn, flip):
                    """dst = ps*cos + swap64(ps)*sin_mod for token block tb."""
                    raw = pA.tile([128, TB], f32, tag="raw", bufs=3, name="raw")
                    nc.scalar.copy(raw[:], ps[:])
                    rot = pA.tile([128, TB], f32, tag="rot", bufs=6, name="rot")
                    nc.gpsimd.dma_start(rot[0:64, :], raw[64:128, :])
                    nc.gpsimd.dma_start(rot[64:128, :], raw[0:64, :])
                    m1 = pA.tile([128, TB], f32, tag="m1", bufs=6, name="m1")
                    nc.vector.tensor_tensor(m1[:], ps[:], cs[:], op=MUL)  # PSUM: DVE
                    nc.vector.tensor_tensor(rot[:], rot[:], sn[:], op=MUL)
                    nc.vector.tensor_tensor(dst_ap, m1[:], rot[:], op=ADD)

                for tb in range(NT):
                    cos_s = pA.tile([128, TB], f32, tag="cos", bufs=4, name="cos")
                    sin_s = pA.tile([128, TB], f32, tag="sin", bufs=4, name="sin")
                    if tb != 0:
                        # rope tables just ahead of the xt tiles
                        nc.sync.dma_start(cos_s[:], cos_d[:, TB * tb:TB * (tb + 1)])
                        nc.sync.dma_start(sin_s[:], sin_d[:, TB * tb:TB * (tb + 1)])
                    xt_t = []
                    for ci in range(NCT):
                        t = pA.tile([128, TB], f32r, tag="xt", bufs=32, name="xt")
                        nc.sync.dma_start(
                            t[:], xt[128 * ci:128 * (ci + 1),
                                     TB * tb:TB * (tb + 1)].bitcast(f32r))
                        xt_t.append(t)
                        if tb == 0 and ci % 4 == 3 and ci < 15:
                            c = ci // 4 + 1  # stream wk in behind the xt tiles
                            nc.sync.dma_start(
                                wk_all[:, 4 * c:4 * (c + 1), :],
                                wk[512 * c:512 * (c + 1), :].bitcast(f32r)
                                .rearrange("(c p) d -> p c d", p=128))
                        if tb == 0 and ci == 0:
                            nc.sync.dma_start(cos_s[:], cos_d[:, 0:TB])
                            nc.sync.dma_start(sin_s[:], sin_d[:, 0:TB])
                        if tb == 0 and ci == 1:
                            id_f = pA.tile([128, 128], f32, tag="idf")
                            nc.sync.dma_start(id_f[:], id_d[:])
                            nc.vector.tensor_copy(ident_b[:], id_f[:])
                            warm = pA.tile([128, 1], f32, tag="warm")
                            nc.scalar.activation(warm[:], id_f[:, 0:1], AF.Exp)
                        if tb == 0 and ci == 10:
                            # wv lands right as the V matmuls want it; K's last
                            # xt tiles shift later but K ends DMA-paced anyway
                            nc.sync.dma_start(wv_all[:], wv[:].bitcast(f32r)
                                              .rearrange("(c p) d -> p c d", p=128))
                    if tb == 0:
                        for h in range(GH):
                            nc.sync.dma_start(
                                wq_all[h][:], wq[:, HD * h:HD * (h + 1)]
                                .bitcast(f32r).rearrange("(c p) d -> p c d", p=128))
                    # K
                    ps_k = psA.tile([128, TB], f32, tag="pk")
                    for ci in range(NCT):
                        nc.tensor.matmul(ps_k[:], wk_all[:, ci, :], xt_t[ci][:],
                                         start=ci == 0, stop=ci == NCT - 1)
                    rope(ps_k, kt_rope[:, TB * tb:TB * (tb + 1)], cos_s, sin_s,
                         flip=True)

                    def emit_v():
                        ps_v = psA.tile([128, TB], f32, tag="pv")
                        for ci in range(NCT):
                            nc.tensor.matmul(ps_v[:], wv_all[:, ci, :],
                                             xt_t[ci][:],
                                             start=ci == 0, stop=ci == NCT - 1)
                        vt_sb = pA.tile([128, TB], bf16, tag="vts", bufs=3,
                                        name="vt_sb")
                        nc.scalar.copy(vt_sb[:], ps_v[:])
                        return vt_sb

                    def emit_vtrans(vt_sb, u):
                        ps_tr = psA.tile([128, 128], bf16, tag="ptr", bufs=1,
                                         name="ps_tr")
                        nc.tensor.transpose(ps_tr[:],
                                            vt_sb[:, 128 * u:128 * (u + 1)],
                                            ident_b[:])
                        # Act (not DVE): DVE's in-order queue sits behind rope
                        # m1 ops that can wait on the cos/sin loads
                        nc.scalar.copy(
                            v_all[:, 128 * (4 * tb + u):128 * (4 * tb + u + 1)],
                            ps_tr[:])

                    vt_sb = emit_v() if tb < NT - 1 else None
                    # Q heads; one V transpose is spread between each pair of
                    # head blocks so the single ptr bank's WAR (on the previous
                    # transpose's drain copy) never stalls the PE
                    for h in range(GH):
                        ps_q = psA.tile([128, TB], f32, tag="pq", bufs=5,
                                        name=f"ps_q{h}")
                        for ci in range(NCT):
                            nc.tensor.matmul(ps_q[:], wq_all[h][:, ci, :],
                                             xt_t[ci][:],
                                             start=ci == 0, stop=ci == NCT - 1)
                        if vt_sb is not None:
                            emit_vtrans(vt_sb, h)
                        rope(ps_q, qt_rope[h][:, TB * tb:TB * (tb + 1)], cos_s,
                             sin_s, flip=(h % 2 == 0))
                    if vt_sb is None:
                        # last block: V after the Q heads, hiding the final
                        # rope chain's latency behind V's matmuls
                        vt_sb = emit_v()
                        for u in range(4):
                            emit_vtrans(vt_sb, u)

            # ---- phase B: attention + partial o_proj (phase-scoped SBUF) ----
            with tc.tile_pool(name="pB", bufs=1) as pB, \
                 tc.tile_pool(name="psB", bufs=1, space="PSUM") as psB:
                # one-time small loads + casts (tiles scoped here, used here)
                cm_f = pB.tile([128, 768], f32, tag="cmf")
                nc.sync.dma_start(cm_f[:], cm_d[:])
                nc.vector.tensor_copy(cm_b[:], cm_f[:])
                on_f = pB.tile([128, 128], f32, tag="onf")
                nc.sync.dma_start(on_f[:], on_d[:])
                nc.vector.tensor_copy(ones_b[:], on_f[:])
                # o_proj weights: first o_proj runs ~25us into phase B, so this
                # load hides behind the first attention block
                wo_all = pB.tile([128, GH * HID], f32r, tag="wo")  # [j-sub, jh*2048+c]
                nc.sync.dma_start(wo_all[:].rearrange("p (h c) -> p h c", h=GH),
                                  wo[:].bitcast(f32r)
                                  .rearrange("(h p) c -> p h c", p=128))

                def oproj_chunk(j, at_j, cb, last=False):
                    """One 4-co-tile chunk of q-block j's o_proj, batched into a
                    single output DMA."""
                    ob4 = pB.tile([128, 4, TB], f32, tag="ob", bufs=4,
                                  name="ob4")
                    for u in range(4):
                        co = 4 * cb + u
                        ps_p = psB.tile([128, TB], f32, tag="ps_d", bufs=2,
                                        name="ps_p")
                        for jh in range(GH):
                            nc.tensor.matmul(ps_p[:],
                                             wo_all[:, jh * HID + 128 * co:
                                                    jh * HID + 128 * (co + 1)],
                                             at_j[jh][:], start=(jh == 0),
                                             stop=(jh == GH - 1))
                        if u % 2 == 1:
                            nc.scalar.copy(ob4[:, u, :], ps_p[:])
                        else:
                            nc.vector.tensor_copy(ob4[:, u, :], ps_p[:])
                    if cb == 0 and not with_rs:
                        # no-collective mode returns oT_part[0:512] as out_r;
                        # write it there directly
                        dst = out_r[:, TB * j:TB * (j + 1)]
                    else:
                        dst = oT_part[512 * cb:512 * (cb + 1),
                                      TB * j:TB * (j + 1)]
                    if last:
                        # two half DMAs so the final transfer tail is shorter
                        d3 = dst.rearrange("(u p) t -> p u t", p=128)
                        nc.sync.dma_start(d3[:, 0:2, :], ob4[:, 0:2, :])
                        nc.sync.dma_start(d3[:, 2:4, :], ob4[:, 2:4, :])
                    else:
                        nc.sync.dma_start(
                            dst.rearrange("(u p) t -> p u t", p=128), ob4[:])

                pending = []  # deferred o_proj chunk closures
                # q-blocks in rotated order so the final (tail) o_proj pairs
                # with the SHORT j=0 block: its at_s chain hides behind the
                # previous block's o_proj instead of stalling the PE
                for j in ([1, 2, 3, 0] if causal else range(NT)):
                    # (i, q-col-offset) tile list: full tiles then tightened diagonal
                    if causal:
                        tiles = [(i, 0) for i in range(4 * j)]
                        tiles += [(4 * j + m, min(128 * m, 256)) for m in range(4)]
                    else:
                        tiles = [(i, 0) for i in range(NKT)]
                    last_i = tiles[-1][0]

                    ps_o = [psB.tile([128, TB], f32, tag="po", bufs=4, name=f"ps_o{h}")
                            for h in range(GH)]
                    acc = [pB.tile([128, TB], bf16, tag=f"acc{h}", bufs=1,
                                   name=f"acch{h}") for h in range(GH)]
                    av_pend = []  # (h, i, off, w, pt) AVs deferred from i<2

                    def flush_av():
                        for (fh, fi, foff, fw, fpt) in av_pend:
                            nc.tensor.matmul(ps_o[fh][:, foff:TB],
                                             v_all[:, 128 * fi:128 * (fi + 1)],
                                             fpt[:, 0:fw],
                                             start=(fi == 0), stop=(fi == last_i),
                                             skip_group_check=True)
                        av_pend.clear()

                    for ti, (i, off) in enumerate(tiles):
                        w = TB - off
                        diag = causal and i >= 4 * j
                        m = i - 4 * j if diag else -1
                        if i == 3:
                            # the i<3 AVs were deferred so the first PSUM write
                            # to the po banks (WAR on last block's at_s) comes
                            # after ~12 scores' worth of PE work
                            flush_av()
                        for h in range(GH):
                            ps_s = psB.tile([128, TB], f32, tag="ps_s", bufs=2,
                                            name="ps_s")
                            nc.tensor.matmul(ps_s[:, 0:w],
                                             kt_rope[:, 128 * i:128 * (i + 1)],
                                             qt_rope[h][:, TB * j + off:TB * (j + 1)],
                                             start=True, stop=True)
                            pt = pB.tile([128, TB], bf16, tag="pt", bufs=20, name="pt")
                            nc.scalar.activation(pt[:, 0:w], ps_s[:, 0:w], AF.Exp)
                            if diag:
                                patt = 512 if m == 3 else 0
                                meng = nc.vector
                                meng.tensor_tensor(
                                    pt[:, 0:w], pt[:, 0:w],
                                    cm_b[:, patt:patt + w], op=MUL)
                            if i == 0:
                                nc.vector.tensor_copy(acc[h][:], pt[:])
                            else:
                                nc.vector.tensor_tensor(acc[h][:, off:TB],
                                                        acc[h][:, off:TB],
                                                        pt[:, 0:w], op=ADD)
                            if i < 3:
                                av_pend.append((h, i, off, w, pt))
                                continue
                            nc.tensor.matmul(ps_o[h][:, off:TB],
                                             v_all[:, 128 * i:128 * (i + 1)],
                                             pt[:, 0:w],
                                             start=False, stop=(i == last_i),
                                             skip_group_check=True)
                    flush_av()
                    # any previous-block o_proj chunks not yet interleaved
                    while pending:
                        pending.pop(0)()

                    # normalize into A^T blocks
                    at_s = [pB.tile([128, TB], f32r, tag=f"at{h}", bufs=1,
                                    name=f"at_s{h}") for h in range(GH)]
                    for h in range(GH):
                        ps_d = psB.tile([128, TB], f32, tag="ps_d", bufs=2,
                                        name="ps_d")
                        nc.tensor.matmul(ps_d[:], ones_b[:], acc[h][:],
                                         start=True, stop=True)
                        rec = pB.tile([128, TB], f32, tag="rec", bufs=2, name="rec")
                        nc.vector.reciprocal(rec[:], ps_d[:])
                        nc.vector.tensor_tensor(at_s[h][:], ps_o[h][:], rec[:],
                                                op=MUL)  # PSUM: DVE only
                    pending = [
                        (lambda jj, aa, cc: lambda la=False: oproj_chunk(
                            jj, aa, cc, la))(j, at_s, cb)
                        for cb in range(NCT // 4)]
                while len(pending) > 2:
                    pending.pop(0)()
                pending.pop(0)(True)
                pending.pop(0)(True)

            if _DEBUG_OUTS:
                dbg_kt = nc.dram_tensor("dbg_kt", [128, S], f32,
                                        kind="ExternalOutput").ap()
                dbg_q0 = nc.dram_tensor("dbg_q0", [128, S], f32,
                                        kind="ExternalOutput").ap()
                dbg_v = nc.dram_tensor("dbg_v", [128, S // 2], f32,
                                       kind="ExternalOutput").ap()
                nc.sync.dma_start(dbg_kt[:], kt_rope[:].bitcast(f32))
                nc.sync.dma_start(dbg_q0[:], qt_rope[0][:].bitcast(f32))
                nc.sync.dma_start(dbg_v[:], v_all[:].bitcast(f32))  # raw bits

            # ---- phase C: ReduceScatter partials, emit this core's slice ----
            if with_rs:
                nc.gpsimd.collective_compute(
                    "ReduceScatter", ADD,
                    replica_groups=[[0, 1, 2, 3], [4, 5, 6, 7]],
                    ins=[oT_part[:].opt()], outs=[oT_red[:].opt()],
                )
                nc.sync.dma_start(out_r[:], oT_red[:])
            # else: emit_oproj already wrote out_r directly

    nc.compile()
    return nc


def kernel(hidden_states, attention_mask, Wq, Wk, Wv, Wo, sin, cos):
    hidden_states = np.asarray(hidden_states, dtype=np.float32)
    attention_mask = np.asarray(attention_mask, dtype=np.float32)
    Wq, Wk, Wv, Wo = (np.ascontiguousarray(np.asarray(a, dtype=np.float32))
                      for a in (Wq, Wk, Wv, Wo))
    sin = np.asarray(sin, dtype=np.float32)
    cos = np.asarray(cos, dtype=np.float32)

    # classify the mask: causal (top-right strictly very-negative, elsewhere 0,
    # col 0 ignored since reference zeroes it) vs all-zeros (full attention)
    m0 = attention_mask[0, 0]
    iu = np.triu_indices(S, k=1)
    causal = bool((m0[iu] < -1e30).all() and
                  (m0[np.tril_indices(S, k=0)] == 0.0).all())
    if not causal:
        assert (attention_mask == 0).all(), "unsupported attention mask pattern"
    if causal:
        for b in range(1, B):
            assert np.array_equal(attention_mask[b, 0], m0), "mask differs per batch"

    key = causal
    if key not in _CACHE:
        _CACHE[key] = _build(causal)
    nc = _CACHE[key]

    cos_t = np.ascontiguousarray(cos[:S].T)          # [128, S]
    sin_t = cos_t.copy()
    sin_t[:] = sin[:S].T
    sin_m = sin_t.copy()
    sin_m[:64] *= -1.0
    # 0/1 causal keep-patterns: patt0 = (q >= k), patt1 = (q >= k + 128)
    kl = np.arange(128)[:, None]
    ql = np.arange(512)[None, :]
    cmask = np.concatenate(
        [(ql >= kl).astype(np.float32),
         (ql[:, :256] >= kl + 128).astype(np.float32)], axis=1)

    in_maps = []
    for c in range(8):
        b, g = c // 4, c % 4
        in_maps.append({
            "xt": np.ascontiguousarray(hidden_states[b].T),
            "wq": np.ascontiguousarray(Wq[512 * g:512 * (g + 1), :].T),
            "wk": np.ascontiguousarray(Wk[128 * g:128 * (g + 1), :].T),
            "wv": np.ascontiguousarray(Wv[128 * g:128 * (g + 1), :].T),
            "wo": np.ascontiguousarray(Wo[:, 512 * g:512 * (g + 1)].T),
            "cos_t": cos_t, "sin_m": sin_m, "cmask": cmask,
            "ones_in": np.ones((128, 128), dtype=np.float32),
            "ident_in": np.eye(128, dtype=np.float32),
        })

    global _LAST_IN_MAPS, _LAST_RES
    _LAST_IN_MAPS = in_maps
    res = run_bass_kernel_spmd(nc, in_maps, core_ids=list(range(8)))
    _LAST_RES = res

    out = np.empty((B, S, HID), dtype=np.float32)
    for c in range(8):
        b, r = c // 4, c % 4
        out[b, :, TB * r:TB * (r + 1)] = res.results[c]["out_r"].T
    return out


if __name__ == "__main__":
    rng = np.random.default_rng(0)
    h = rng.standard_normal((B, S, HID), dtype=np.float32)
    print("module loads ok")
